# revision 77
# baseline (speedup 1.0000x reference)
"""Self-contained Trainium2 Bass kernel for the dense transformer block.

Head-parallel attention + half-chunk token ownership:
 - Each of the 8 cores computes Q/K/V + causal attention for 2 of the 16
   heads over BOTH batch elements.
 - After batch b's attention, an 8-rank AllToAll redistributes the
   (unnormalized) attention outputs + softmax denominators so core c ends
   up owning tokens [c*256,(c+1)*256) of EVERY batch: 256 tokens from
   batch 0 (via A2A#0) and 256 from batch 1 (via A2A#1).
 - proj+LN1 run per 256-token half as soon as that half's A2A lands;
   FFN1 also runs per half (so the batch-0 half of the FFN overlaps
   A2A#1), while FFN2 + LN2 run once on the combined 512 columns.
Other key tricks vs a straightforward port:
 - exp() is the scalar-engine bottleneck of attention, so scores are
   staged 4 k-tiles at a time into one [128,4096] SBUF tile and a single
   ACTIVATE covers them (amortizes the ~350-cycle ACT pipeline fill and
   the per-instruction semaphore wait).
 - batch-1 QKV matmuls are pumped one-at-a-time into batch-0 attention's
   tensor-engine idle slots.
 - PE warmup matmuls at t=0 lift the HAM clock gate before real work.
 - masks / identity / head-select matrices are precomputed on host.
 - 1/x and 1/sqrt(x) computed as exp(-ln x) / exp(-0.5 ln x) on the
   scalar engine (vector reciprocal is ~6.5ns/elem/lane).
"""
import sys as _sys
if "/opt/trn_rl_repo" not in _sys.path:
    _sys.path.insert(0, "/opt/trn_rl_repo")

import numpy as np
import ml_dtypes

import concourse.bass as bass
import concourse.tile as tile
from concourse import bacc, mybir

F32 = mybir.dt.float32
BF16 = mybir.dt.bfloat16
AF = mybir.ActivationFunctionType
ALU = mybir.AluOpType

B, T, C, H, HS, FF = 2, 2048, 1024, 16, 64, 4096
TL = 512               # output columns per core (256 from each batch)
HT = 256               # tokens per core per batch
NCT = C // 128         # 8 feature tiles
NFT = FF // 128        # 32 ff tiles
EPS = 1e-5
N_CORES = 8
VW = HS + 1            # 65: [v | ones] block per head
GB = 130               # a2a block rows: 128 attn feats + 2 denominators
SC = 0.125             # 1/sqrt(HS)


def build_program():
    nc = bacc.Bacc("TRN2", target_bir_lowering=False, debug=False,
                   enable_asserts=False, num_devices=N_CORES)

    d = {}
    d["xb"] = nc.dram_tensor("xb", (C, B * T), BF16, kind="ExternalInput").ap()
    d["xloc"] = nc.dram_tensor("xloc", (C, TL), F32, kind="ExternalInput").ap()
    d["wqkv"] = nc.dram_tensor("wqkv", (C, 384), BF16,
                               kind="ExternalInput").ap()
    d["wproj"] = nc.dram_tensor("wproj", (C, C), BF16,
                                kind="ExternalInput").ap()
    d["w1"] = nc.dram_tensor("w1", (C, FF), BF16, kind="ExternalInput").ap()
    d["w2"] = nc.dram_tensor("w2", (FF, C), BF16, kind="ExternalInput").ap()
    # bias/ln vectors arrive p-major [128, len/128] (host pre-transposes)
    for name, n in [("bproj", NCT), ("b1", NFT), ("b2", NCT), ("ln1g", NCT),
                    ("ln1b", NCT), ("ln2g", NCT), ("ln2b", NCT)]:
        d[name] = nc.dram_tensor(name, (128, n), F32,
                                 kind="ExternalInput").ap()
    d["maskpack"] = nc.dram_tensor("maskpack", (128, 4096), BF16,
                                   kind="ExternalInput").ap()
    d["ident"] = nc.dram_tensor("ident", (128, 128), BF16,
                                kind="ExternalInput").ap()
    d["selpack"] = nc.dram_tensor("selpack", (16, 8 * 128), BF16,
                                  kind="ExternalInput").ap()
    d["out"] = nc.dram_tensor("out", (C, TL), F32, kind="ExternalOutput").ap()

    with tile.TileContext(nc) as tc:
        _emit(tc, d)

    nc.compile()
    return nc


def _emit(tc, d):
    nc = tc.nc
    dmaq = [nc.sync, nc.scalar, nc.gpsimd]

    # ---------------- constants / small inputs ----------------
    const = tc.alloc_tile_pool(name="const", bufs=1)

    ident = const.tile([128, 128], BF16, tag="ident")
    nc.sync.dma_start(ident[:], d["ident"])

    # NOTE: the scalar queue carries NO DMAs until the FFN phases — a DMA
    # parked there (ring-credit waits) stalls every exp behind it.
    # Bias DMAs are emitted after the first x stripes (see below).
    bias_sb = {}
    for name in ("bproj", "b1", "b2", "ln1g", "ln1b", "ln2g", "ln2b"):
        n = d[name].shape[1]            # host passes p-major [128, n]
        t_ = const.tile([128, n], F32, tag=name, name=name)
        bias_sb[name] = t_
    bproj_sb, b1_sb, b2_sb = bias_sb["bproj"], bias_sb["b1"], bias_sb["b2"]
    ln1g_sb, ln1b_sb = bias_sb["ln1g"], bias_sb["ln1b"]
    ln2g_sb, ln2b_sb = bias_sb["ln2g"], bias_sb["ln2b"]

    ones_f32 = const.tile([128, 1], F32, tag="ones_f32")       # LN col-sum
    nc.gpsimd.memset(ones_f32[:], 1.0)
    eps_sb = const.tile([1, 1], F32, tag="eps")
    nc.gpsimd.memset(eps_sb[:], EPS)
    onesc = const.tile([1, 128], F32, tag="onesc")   # row-broadcast lhsT
    nc.gpsimd.memset(onesc[:], 1.0)
    maskpack = const.tile([128, 4096], BF16, tag="maskpack")
    selpack = const.tile([16, 8 * 128], BF16, tag="selpack")

    # ---------------- activation storage ----------------
    xloc_pool = tc.alloc_tile_pool(name="xloc_pool", bufs=1)
    xloc = [xloc_pool.tile([128, TL], F32, tag=f"xloc{i}", name=f"xloc{i}")
            for i in range(NCT)]
    wproj_pool = tc.alloc_tile_pool(name="wproj", bufs=1)
    wproj_sb = [wproj_pool.tile([128, C], BF16, tag=f"wp{i}", name=f"wp{i}")
                for i in range(NCT)]

    kqv_pool = tc.alloc_tile_pool(name="kqv_pool", bufs=1)
    q_sb = kqv_pool.tile([128, B * T], BF16, tag="q", name="q")
    k_sb = kqv_pool.tile([128, B * T], BF16, tag="k", name="k")
    v_sb = [kqv_pool.tile([128, 2 * VW], BF16, tag=f"v{j}", name=f"v{j}")
            for j in range(2 * (T // 128))]                    # 32 k-tiles
    wqkv_sb = [kqv_pool.tile([128, 384], BF16, tag=f"wqkv{i}",
                             name=f"wqkv{i}") for i in range(NCT)]
    vf_sb = kqv_pool.tile([128, T], BF16, tag="vf", name="vf")

    # right-side pools, bottom -> top (LIFO release order: top first)
    raw_pool = tc.alloc_tile_pool(name="raw_pool", bufs=1, side="right")
    attn_raw = [[raw_pool.tile([128, HT], BF16, tag=f"ar{b}_{s}",
                               name=f"ar{b}_{s}") for s in range(N_CORES)]
                for b in range(B)]
    den_raw = [raw_pool.tile([16, HT], BF16, tag=f"dr{b}", name=f"dr{b}")
               for b in range(B)]
    xbf1_pool = tc.alloc_tile_pool(name="xbf1_pool", bufs=1, side="right")
    xbf1 = [xbf1_pool.tile([128, T], BF16, tag=f"xb1_{i}", name=f"xb1_{i}")
            for i in range(NCT)]
    xbf0_pool = tc.alloc_tile_pool(name="xbf0_pool", bufs=1, side="right")
    xbf0 = [xbf0_pool.tile([128, T], BF16, tag=f"xb0_{i}", name=f"xb0_{i}")
            for i in range(NCT)]
    xbf = [xbf0, xbf1]

    # ---------------- input DMA, priority order ----------------
    # startup is HBM-bound AND the first exp waits on chunk-0's QKV: wqkv
    # then chunk-0/1 x stripes go out FIRST; everything else after.
    for i in range(NCT):
        [nc.sync, nc.gpsimd][i % 2].dma_start(
            wqkv_sb[i][:], d["wqkv"][i * 128:(i + 1) * 128, :])
    qi = 0

    def x_stripes(b, tcol):
        nonlocal qi
        for i in range(NCT):
            c0 = tcol * 1024
            [nc.sync, nc.gpsimd][qi % 2].dma_start(
                xbf[b][i][:, c0:c0 + 1024],
                d["xb"][i * 128:(i + 1) * 128,
                        b * T + c0:b * T + c0 + 1024])
            qi += 1

    x_stripes(0, 0)                             # chunks 0-1: unblock attn
    nc.sync.dma_start(maskpack[:], d["maskpack"])
    for k_, name in enumerate(bias_sb):
        [nc.gpsimd, nc.sync][k_ % 2].dma_start(bias_sb[name][:], d[name])
    x_stripes(0, 1)
    nc.sync.dma_start(selpack[:], d["selpack"])
    x_stripes(1, 0)
    x_stripes(1, 1)

    # a2a DRAM bounce buffers, no batch padding: group g on core p holds
    # p's 2 heads (+2 denom rows) for tokens [g*256,(g+1)*256) of batch b
    dram = tc.alloc_tile_pool(name="dram", bufs=1, space="DRAM")
    a2a_in = [dram.tile([N_CORES * GB, HT], BF16, tag=f"a2a_in{b}",
                        name=f"a2a_in{b}") for b in range(B)]
    a2a_out = [dram.tile([N_CORES * GB, HT], BF16, tag=f"a2a_out{b}",
                         name=f"a2a_out{b}") for b in range(B)]

    # the [v | ones] blocks' ones columns never change: fill them all now
    # (on vector — it is idle at startup; gpsimd is busy with x stripes)
    for j in range(2 * (T // 128)):
        vj = v_sb[j].rearrange("p (h w) -> p h w", w=VW)
        nc.vector.memset(vj[:, :, HS:VW], 1.0)

    # ---------------- PE clock warmup + keep-warm fillers ----------------
    # The HAM clock gate halves the PE clock after ~3.4us of idle and needs
    # ~3.4us of sustained activity to lift again. Dependency-free standalone
    # LDWEIGHTS on the identity (no PSUM, no consumers) emitted at known
    # bubble points keep the activity monitor busy.
    def filler(n):
        for _ in range(n):
            nc.tensor.ldweights(ident[:])

    with tc.tile_pool(name="warmps", bufs=1, space="PSUM") as wpool:
        wps = wpool.tile([128, 128], F32, tag="wps", name="wps")
        NWARM = 8
        for i in range(NWARM):
            nc.tensor.matmul(wps[:], ident[:], ident[:],
                             start=(i == 0), stop=(i == NWARM - 1))

    # ------------- P1+P2, chunk-interleaved, per batch + A2A -------------
    p2sb = tc.alloc_tile_pool(name="p2sb", bufs=1)
    warm = p2sb.tile([1, 1], F32, tag="warm", name="warm")
    nc.scalar.activation(warm[:], eps_sb[:], AF.Exp)

    p1ps = tc.alloc_tile_pool(name="p1ps", bufs=1, space="PSUM")    # 1 bank
    p1vps = tc.alloc_tile_pool(name="p1vps", bufs=1, space="PSUM")  # 1 bank
    p2ps = tc.alloc_tile_pool(name="p2ps", bufs=1, space="PSUM")    # 4 banks
    avps = tc.alloc_tile_pool(name="avps", bufs=1, space="PSUM")    # 2 banks

    def qkv_mms(b, tch):
        """Q/K/V matmuls for one 512-token chunk of batch b.

        Generator: yields after each matmul (True at part boundaries) so
        the caller can interleave these into attention's tensor idle
        slots. Transposes are NOT included — a PE transpose emitted while
        an attention accumulation group is open corrupts it.
        """
        for cols, dst, off in (
                (slice(0, 128), q_sb, b * T + tch * 512),
                (slice(128, 256), k_sb, b * T + tch * 512),
                (slice(256, 384), vf_sb, tch * 512)):
            ps = p1ps.tile([128, 512], F32, tag="p1", bufs=1, name="p1")
            for c in range(NCT):
                nc.tensor.matmul(ps[:], wqkv_sb[c][:, cols],
                                 xbf[b][c][:, tch * 512:(tch + 1) * 512],
                                 start=(c == 0), stop=(c == NCT - 1))
                if c < NCT - 1:
                    yield False
            nc.vector.tensor_copy(dst[:, off:off + 512], ps[:])
            yield True

    def v_transposes(b, tch):
        for kk in range(4):
            kt = 16 * b + 4 * tch + kk
            ps2 = p1vps.tile([128, 128], BF16, tag="p1v", bufs=1, name="p1v")
            nc.tensor.transpose(
                ps2[:], vf_sb[:, (4 * tch + kk) * 128:(4 * tch + kk + 1) * 128],
                ident[:])
            vj = v_sb[kt].rearrange("p (h w) -> p h w", w=VW)
            nc.vector.tensor_copy(
                vj[:, :, 0:HS], ps2[:].rearrange("p (h w) -> p h w", w=HS))

    # one global QKV stream: chunk (b,tch) = id 4b+tch; id 0 emitted
    # inline, ids 1..7 pumped into attention idle slots ACROSS batches
    # (batch 1's first chunks stream in during batch 0's last q-chunk).
    prog = [0]                                # highest fully-emitted id
    safe = [True]                             # gen at a part boundary?

    def qkv_all():
        for cid in range(1, 8):
            yield from qkv_mms(cid // 4, cid % 4)
            prog[0] = cid

    gen = qkv_all()

    def pump(n):
        for _ in range(n):
            r = next(gen, None)
            if r is None:
                prog[0] = 7
                safe[0] = True
                filler(1)
            else:
                safe[0] = r

    def attn_batch(b):
        """QKV + causal attention for the 2 local heads + A2A of batch b."""
        if b == 0:
            for _ in qkv_mms(0, 0):
                pass
        for j in range(4):                    # local q-chunks of 512
            while prog[0] < 4 * b + j or not safe[0]:
                pump(1)                       # chunk j emitted, group closed
            q0 = b * T + j * 512
            nkt = 4 * j + 4                   # causal k-tiles
            avs = [avps.tile([VW, 512], F32, tag=f"av{hh}", bufs=1,
                             name=f"av{hh}") for hh in range(2)]
            sc_t = [None] * nkt

            def emit_sc(kt):
                sc = p2ps.tile([128, 1024], F32, tag="sc", bufs=2,
                               name="sc")
                for hh, po in ((0, 0), (1, 64)):
                    nc.tensor.matmul(
                        sc[:, hh * 512:(hh + 1) * 512],
                        k_sb[po:po + HS,
                             b * T + kt * 128:b * T + (kt + 1) * 128],
                        q_sb[po:po + HS, q0:q0 + 512],
                        start=True, stop=True, tile_position=(po, 0))
                sc_t[kt] = sc

            emit_sc(0)
            # chunk j's v-transposes: after sc(kt0) so the first exp isn't
            # delayed, before av(kt0) so no accumulation group is open
            v_transposes(b, j)
            for kt in range(nkt):
                probs = p2sb.tile([128, 1024], BF16, tag="probs", bufs=6,
                                  name="probs")
                nc.scalar.activation(probs[:], sc_t[kt][:], AF.Exp, scale=SC)
                sc_t[kt] = None
                if kt + 1 < nkt:
                    emit_sc(kt + 1)
                if kt >= nkt - 4:             # diagonal k-tile: causal mask
                    i = kt - (nkt - 4)
                    nc.vector.tensor_mul(
                        probs[:], probs[:],
                        maskpack[:, i * 1024:(i + 1) * 1024])
                for hh in range(2):
                    nc.tensor.matmul(
                        avs[hh][:],
                        v_sb[b * 16 + kt][:, hh * VW:(hh + 1) * VW],
                        probs[:, hh * 512:(hh + 1) * 512],
                        start=(kt == 0), stop=(kt == nkt - 1))
                pump(2)
            # stage unnormalized attn + denominators into the A2A src.
            # bufs=4: every j gets its own slot, so this copy never waits
            # on staging DMAs that may be parked behind an in-flight A2A.
            for hh in range(2):
                sth = p2sb.tile([VW, 512], BF16, tag=f"st{hh}", bufs=4,
                                name=f"st{hh}")
                nc.vector.tensor_copy(sth[:], avs[hh][:])
                for s_ in range(2):
                    g = 2 * j + s_
                    csl = slice(s_ * HT, (s_ + 1) * HT)
                    nc.gpsimd.dma_start(
                        a2a_in[b][g * GB + hh * HS:g * GB + (hh + 1) * HS,
                                  :], sth[0:HS, csl])
                    nc.gpsimd.dma_start(
                        a2a_in[b][g * GB + 128 + hh:g * GB + 129 + hh, :],
                        sth[HS:VW, csl])
            pump(3)
        for _ in gen:                         # drain any leftover QKV
            pass
        nc.gpsimd.collective_compute(
            "AllToAll", ALU.bypass,
            replica_groups=[list(range(N_CORES))],
            ins=[a2a_in[b][:].opt()], outs=[a2a_out[b][:].opt()])

    attn_batch(0)
    xbf0_pool.release()
    # deferred loads the later phases need (HBM was saturated until here)
    for i in range(NCT):
        [nc.sync, nc.gpsimd][i % 2].dma_start(
            xloc[i][:], d["xloc"][i * 128:(i + 1) * 128, :])
    for i in range(NCT):
        [nc.sync, nc.gpsimd][i % 2].dma_start(
            wproj_sb[i][:], d["wproj"][i * 128:(i + 1) * 128, :])
    attn_batch(1)
    xbf1_pool.release()
    avps.release()
    p2ps.release()
    p1vps.release()
    p1ps.release()
    p2sb.release()
    kqv_pool.release()

    # ---------------- P3/P4 ----------------
    x2_pool = tc.alloc_tile_pool(name="x2_pool", bufs=1)
    x2b = [x2_pool.tile([128, TL], BF16, tag=f"x2b{i}", name=f"x2b{i}")
           for i in range(NCT)]
    h_pool = tc.alloc_tile_pool(name="h_pool", bufs=1)
    h_sb = [h_pool.tile([128, TL], BF16, tag=f"h{i}", name=f"h{i}")
            for i in range(NFT)]
    r2_pool = tc.alloc_tile_pool(name="r2_pool", bufs=1)
    resid2 = [r2_pool.tile([128, TL], F32, tag=f"r2_{i}", name=f"r2_{i}")
              for i in range(NCT)]

    # w1 resident (8MB) in the space freed by xbf; streamed in now
    w1res = tc.alloc_tile_pool(name="w1res", bufs=1, side="right")
    w1r = [w1res.tile([128, FF], BF16, tag=f"w1r{i}", name=f"w1r{i}")
           for i in range(NCT)]
    for qt in range(4):                       # quarter-major for FFN1a order
        for i in range(NCT):
            [nc.sync, nc.gpsimd][(qt * NCT + i) % 2].dma_start(
                w1r[i][:, qt * 1024:(qt + 1) * 1024],
                d["w1"][i * 128:(i + 1) * 128, qt * 1024:(qt + 1) * 1024])

    def p3_half(hf):
        """Normalize + project + residual + LN1 for one 256-token half."""
        # later logical phase: keep the scheduler from hoisting this
        # half's collective-dependent vector ops into earlier streams
        tc.cur_priority += 1
        cols = slice(hf * HT, (hf + 1) * HT)
        # gather this half's A2A result (the sync queue parks here on the
        # collective-done semaphore, so this is emitted as late as possible)
        for p in range(N_CORES):
            nc.sync.dma_start(attn_raw[hf][p][:],
                              a2a_out[hf][p * GB:p * GB + 128, :])
            nc.sync.dma_start(den_raw[hf][2 * p:2 * p + 2, :],
                              a2a_out[hf][p * GB + 128:(p + 1) * GB, :])
        filler(8)
        with tc.tile_pool(name=f"p3sb{hf}", bufs=1) as sb, \
             tc.tile_pool(name=f"p3ps{hf}", bufs=1, space="PSUM") as ps:
            rcpf = sb.tile([16, HT], F32, tag="rcpf", name="rcpf")
            nc.vector.reciprocal(rcpf[:], den_raw[hf][:])
            rcp16 = sb.tile([16, HT], BF16, tag="rcp16", name="rcp16")
            nc.vector.tensor_copy(rcp16[:], rcpf[:])
            attn_n = [sb.tile([128, HT], BF16, tag=f"an{s}", name=f"an{s}")
                      for s in range(N_CORES)]
            for s in range(N_CORES):
                bcp = ps.tile([128, HT], F32, tag="bc", bufs=2, name="bcp")
                nc.tensor.matmul(bcp[:], selpack[:, s * 128:(s + 1) * 128],
                                 rcp16[:], start=True, stop=True)
                nc.vector.tensor_mul(attn_n[s][:], attn_raw[hf][s][:],
                                     bcp[:])
                filler(1)

            resid1 = [sb.tile([128, HT], F32, tag=f"r1_{e}", name=f"r1_{e}")
                      for e in range(NCT)]
            mu_t = ps.tile([1, HT], F32, tag="mu", bufs=1, name="mu_t")
            sq_t = ps.tile([1, HT], F32, tag="sq", bufs=1, name="sq_t")
            mu_ps, sq_ps = mu_t[:], sq_t[:]
            for e in range(NCT):
                pr = ps.tile([128, HT], F32, tag="pr", bufs=2, name="pr")
                for s in range(NCT):
                    nc.tensor.matmul(
                        pr[:], wproj_sb[s][:, e * 128:(e + 1) * 128],
                        attn_n[s][:], start=(s == 0), stop=(s == NCT - 1))
                sa = sb.tile([128, HT], F32, tag="sa", bufs=2, name="sa")
                nc.vector.tensor_scalar_add(sa[:], pr[:], bproj_sb[:, e:e + 1])
                nc.gpsimd.tensor_add(resid1[e][:], sa[:], xloc[e][:, cols])
                # LN1 statistics, interleaved
                nc.tensor.matmul(mu_ps, ones_f32[:], resid1[e][:],
                                 start=(e == 0), stop=(e == NCT - 1))
                sqt = sb.tile([128, HT], F32, tag="sqt", bufs=2, name="sqt")
                eng = nc.vector if e % 2 else nc.gpsimd
                eng.tensor_mul(sqt[:], resid1[e][:], resid1[e][:])
                nc.tensor.matmul(sq_ps, ones_f32[:], sqt[:],
                                 start=(e == 0), stop=(e == NCT - 1))
                filler(2)
            # LN1 scalar chain on [1, 256]
            mu = sb.tile([1, HT], F32, tag="lnmu", name="lnmu")
            nc.scalar.activation(mu[:], mu_ps, AF.Identity, scale=1.0 / C)
            mu2 = sb.tile([1, HT], F32, tag="lnmu2", name="lnmu2")
            nc.scalar.square(mu2[:], mu[:])
            ms = sb.tile([1, HT], F32, tag="lnms", name="lnms")
            nc.scalar.activation(ms[:], sq_ps, AF.Identity, scale=1.0 / C)
            var = sb.tile([1, HT], F32, tag="lnvar", name="lnvar")
            nc.vector.tensor_sub(var[:], ms[:], mu2[:])
            sd = sb.tile([1, HT], F32, tag="lnsd", name="lnsd")
            nc.scalar.activation(sd[:], var[:], AF.Sqrt, bias=eps_sb[:])
            rstd = sb.tile([1, HT], F32, tag="lnrstd", name="lnrstd")
            nc.vector.reciprocal(rstd[:], sd[:])
            filler(16)
            mu_bcp = ps.tile([128, HT], F32, tag="bc", bufs=2, name="mubc")
            nc.tensor.matmul(mu_bcp[:], onesc[:], mu[:], start=True,
                             stop=True)
            rs_bcp = ps.tile([128, HT], F32, tag="bc", bufs=2, name="rsbc")
            nc.tensor.matmul(rs_bcp[:], onesc[:], rstd[:], start=True,
                             stop=True)
            for e in range(NCT):
                t1 = sb.tile([128, HT], F32, tag="t1", bufs=3, name="t1")
                nc.vector.tensor_sub(t1[:], resid1[e][:], mu_bcp[:])
                t2 = sb.tile([128, HT], F32, tag="t2", bufs=3, name="t2")
                nc.vector.tensor_mul(t2[:], t1[:], rs_bcp[:])
                # g*x + b on the (idle) scalar engine: per-partition scale
                nc.scalar.activation(x2b[e][:, cols], t2[:], AF.Identity,
                                     bias=ln1b_sb[:, e:e + 1],
                                     scale=ln1g_sb[:, e:e + 1])
                filler(3)

    def ffn1_half(hf):
        cols = slice(hf * HT, (hf + 1) * HT)
        with tc.tile_pool(name=f"f1ps{hf}", bufs=1, space="PSUM") as ps:
            for f in range(NFT):
                hp = ps.tile([128, HT], F32, tag="h1", bufs=4, name="h1")
                for c in range(NCT):
                    nc.tensor.matmul(hp[:], w1r[c][:, f * 128:(f + 1) * 128],
                                     x2b[c][:, cols],
                                     start=(c == 0), stop=(c == NCT - 1))
                nc.vector.tensor_scalar(h_sb[f][:, cols], hp[:],
                                        b1_sb[:, f:f + 1], 0.0,
                                        op0=ALU.add, op1=ALU.max)

    p3_half(0)
    ffn1_half(0)       # overlaps A2A#1
    p3_half(1)
    ffn1_half(1)
    w1res.release()

    # ---------------- FFN2 + LN2 ----------------
    p4w = tc.alloc_tile_pool(name="p4w_pool", bufs=1, side="right")
    statps = tc.alloc_tile_pool(name="statps", bufs=1, space="PSUM")
    mu2_ps = statps.tile([1, TL], F32, tag="mu2", name="mu2")
    sq2_ps = statps.tile([1, TL], F32, tag="sq2", name="sq2")
    with tc.tile_pool(name="p4sb", bufs=1) as sb4, \
         tc.tile_pool(name="p4ps", bufs=1, space="PSUM") as ps4:
        for ei in range(8):                    # eighths of FF
            w2e = [p4w.tile([128, C], BF16, tag=f"w2e{i}", bufs=2,
                            name=f"w2e{i}") for i in range(4)]
            for i in range(4):
                f = ei * 4 + i
                [nc.sync, nc.gpsimd][i % 2].dma_start(
                    w2e[i][:], d["w2"][f * 128:(f + 1) * 128, :])
            for e in range(NCT):
                ff = ps4.tile([128, TL], F32, tag="ff", bufs=3, name="ff")
                for i in range(4):
                    nc.tensor.matmul(ff[:], w2e[i][:, e * 128:(e + 1) * 128],
                                     h_sb[ei * 4 + i][:],
                                     start=(i == 0), stop=(i == 3))
                if ei == 0:
                    tmp = sb4.tile([128, TL], F32, tag="ft", bufs=3,
                                   name="ft")
                    nc.scalar.activation(tmp[:], ff[:], AF.Identity,
                                         bias=b2_sb[:, e:e + 1])
                    nc.vector.tensor_add(resid2[e][:], tmp[:], x2b[e][:])
                else:
                    nc.vector.tensor_add(resid2[e][:], resid2[e][:], ff[:])
                if ei == 7:
                    # LN2 statistics interleave with the last FFN2 pass
                    nc.tensor.matmul(mu2_ps[:], ones_f32[:], resid2[e][:],
                                     start=(e == 0), stop=(e == NCT - 1))
                    sq2t = sb4.tile([128, TL], F32, tag="sq2t", bufs=2,
                                    name="sq2t")
                    nc.scalar.square(sq2t[:], resid2[e][:])
                    nc.tensor.matmul(sq2_ps[:], ones_f32[:], sq2t[:],
                                     start=(e == 0), stop=(e == NCT - 1))
                    filler(4)

    # ---------------- LN2 + output ----------------
    tc.cur_priority += 1
    with tc.tile_pool(name="p5sb", bufs=1) as sb5, \
         tc.tile_pool(name="p5ps", bufs=1, space="PSUM") as ps5:
        mu = sb5.tile([1, TL], F32, tag="lnmu", name="lnmu")
        nc.scalar.activation(mu[:], mu2_ps[:], AF.Identity, scale=1.0 / C)
        mu2 = sb5.tile([1, TL], F32, tag="lnmu2", name="lnmu2")
        nc.scalar.square(mu2[:], mu[:])
        ms = sb5.tile([1, TL], F32, tag="lnms", name="lnms")
        nc.scalar.activation(ms[:], sq2_ps[:], AF.Identity, scale=1.0 / C)
        var = sb5.tile([1, TL], F32, tag="lnvar", name="lnvar")
        nc.vector.tensor_sub(var[:], ms[:], mu2[:])
        sd = sb5.tile([1, TL], F32, tag="lnsd", name="lnsd")
        nc.scalar.activation(sd[:], var[:], AF.Sqrt, bias=eps_sb[:])
        rstd = sb5.tile([1, TL], F32, tag="lnrstd", name="lnrstd")
        nc.vector.reciprocal(rstd[:], sd[:])
        filler(24)
        mu_bcp = ps5.tile([128, TL], F32, tag="mubc", bufs=1, name="mubc")
        nc.tensor.matmul(mu_bcp[:], onesc[:], mu[:], start=True, stop=True)
        rs_bcp = ps5.tile([128, TL], F32, tag="rsbc", bufs=1, name="rsbc")
        nc.tensor.matmul(rs_bcp[:], onesc[:], rstd[:], start=True, stop=True)
        for e in range(NCT):
            t1 = sb5.tile([128, TL], F32, tag="t1", bufs=3, name="t1")
            nc.vector.tensor_sub(t1[:], resid2[e][:], mu_bcp[:])
            t2 = sb5.tile([128, TL], F32, tag="t2", bufs=3, name="t2")
            nc.vector.tensor_mul(t2[:], t1[:], rs_bcp[:])
            of = sb5.tile([128, TL], F32, tag="of", bufs=3, name="of")
            nc.scalar.activation(of[:], t2[:], AF.Identity,
                                 bias=ln2b_sb[:, e:e + 1],
                                 scale=ln2g_sb[:, e:e + 1])
            [nc.sync, nc.gpsimd][e % 2].dma_start(
                d["out"][e * 128:(e + 1) * 128, :], of[:])

    statps.release()
    r2_pool.release()
    h_pool.release()
    x2_pool.release()
    p4w.release()
    wproj_pool.release()
    raw_pool.release()
    dram.release()
    xloc_pool.release()
    const.release()


# ---------------- host side ----------------

def host_prepare(x, wq, wk, wv, wproj, bproj, ln1_g, ln1_b, w1, b1, w2, b2,
                 ln2_g, ln2_b):
    bf = ml_dtypes.bfloat16
    xT = np.concatenate([np.ascontiguousarray(x[b].T) for b in range(B)],
                        axis=1)                       # [C, B*T] fp32
    # causal masks for diagonal k-tiles, each duplicated for the 2 heads:
    # block i (cols [i*1024,(i+1)*1024)) = [m_i | m_i],
    # m_i[p, t] = 1 iff i*128 + p <= t
    p = np.arange(128)[:, None]
    t = np.arange(512)[None, :]
    mp = np.concatenate(
        [np.tile((128 * i + p <= t).astype(np.float32), (1, 2))
         for i in range(4)], axis=1)
    # selpack: sel_s[r, q] = 1 iff r == 2s + q//64 (head-denominator
    # broadcast: bcp[q, t] = rcp[2s + q//64, t])
    selpack = np.zeros((16, 8 * 128), np.float32)
    for s in range(8):
        for blk in range(2):
            selpack[2 * s + blk, s * 128 + blk * 64:s * 128 + (blk + 1) * 64] = 1
    def pmaj(v):     # [n*128] -> [128, n], row p holds v[p::128]... v[a*128+p]
        return np.ascontiguousarray(
            v.reshape(-1, 128).T).astype(np.float32)

    shared = {
        "xb": xT.astype(bf),
        "wproj": wproj.astype(bf),
        "w1": w1.astype(bf),
        "w2": w2.astype(bf),
        "bproj": pmaj(bproj),
        "b1": pmaj(b1),
        "b2": pmaj(b2),
        "ln1g": pmaj(ln1_g),
        "ln1b": pmaj(ln1_b),
        "ln2g": pmaj(ln2_g),
        "ln2b": pmaj(ln2_b),
        "maskpack": mp.astype(bf),
        "ident": np.eye(128).astype(bf),
        "selpack": selpack.astype(bf),
    }
    in_maps = []
    for core in range(N_CORES):
        h0 = 2 * core
        wqkv = np.concatenate(
            [wq[h0], wq[h0 + 1], wk[h0], wk[h0 + 1], wv[h0], wv[h0 + 1]],
            axis=1).astype(bf)                        # [C, 384]
        xloc = np.concatenate(
            [xT[:, b * T + core * HT: b * T + (core + 1) * HT]
             for b in range(B)], axis=1).astype(np.float32)   # [C, 512]
        in_maps.append({"wqkv": wqkv, "xloc": np.ascontiguousarray(xloc),
                        **shared})
    return in_maps


def host_finalize(results):
    out = np.empty((B, T, C), np.float32)
    for core in range(N_CORES):
        r = results[core]["out"]
        for b in range(B):
            out[b, core * HT:(core + 1) * HT, :] = \
                r[:, b * HT:(b + 1) * HT].T.astype(np.float32)
    return out


# ---------------- top-level entry ----------------
from concourse.bass_utils import run_bass_kernel_spmd as _run_spmd

_nc_cache = None


def _program():
    global _nc_cache
    if _nc_cache is None:
        _nc_cache = build_program()
    return _nc_cache


def run(inputs, trace=False):
    nc = _program()
    in_maps = host_prepare(**inputs)
    res = _run_spmd(nc, in_maps, core_ids=list(range(N_CORES)), trace=trace)
    return host_finalize(res.results), res


def kernel(**inputs):
    out, _ = run(inputs, trace=False)
    return out


# revision 78
# speedup vs baseline: 1.0618x; 1.0618x over previous
"""Self-contained Trainium2 Bass kernel for the dense transformer block.

Head-parallel attention + half-chunk token ownership:
 - Each of the 8 cores computes Q/K/V + causal attention for 2 of the 16
   heads over BOTH batch elements.
 - After batch b's attention, an 8-rank AllToAll redistributes the
   (unnormalized) attention outputs + softmax denominators so core c ends
   up owning tokens [c*256,(c+1)*256) of EVERY batch: 256 tokens from
   batch 0 (via A2A#0) and 256 from batch 1 (via A2A#1).
 - proj+LN1 run per 256-token half as soon as that half's A2A lands;
   FFN1 also runs per half (so the batch-0 half of the FFN overlaps
   A2A#1), while FFN2 + LN2 run once on the combined 512 columns.
Other key tricks vs a straightforward port:
 - exp() is the scalar-engine bottleneck of attention, so scores are
   staged 4 k-tiles at a time into one [128,4096] SBUF tile and a single
   ACTIVATE covers them (amortizes the ~350-cycle ACT pipeline fill and
   the per-instruction semaphore wait).
 - batch-1 QKV matmuls are pumped one-at-a-time into batch-0 attention's
   tensor-engine idle slots.
 - PE warmup matmuls at t=0 lift the HAM clock gate before real work.
 - masks / identity / head-select matrices are precomputed on host.
 - 1/x and 1/sqrt(x) computed as exp(-ln x) / exp(-0.5 ln x) on the
   scalar engine (vector reciprocal is ~6.5ns/elem/lane).
"""
import sys as _sys
if "/opt/trn_rl_repo" not in _sys.path:
    _sys.path.insert(0, "/opt/trn_rl_repo")

import numpy as np
import ml_dtypes

import concourse.bass as bass
import concourse.tile as tile
from concourse import bacc, mybir

F32 = mybir.dt.float32
BF16 = mybir.dt.bfloat16
AF = mybir.ActivationFunctionType
ALU = mybir.AluOpType

B, T, C, H, HS, FF = 2, 2048, 1024, 16, 64, 4096
TL = 512               # output columns per core (256 from each batch)
HT = 256               # tokens per core per batch
NCT = C // 128         # 8 feature tiles
NFT = FF // 128        # 32 ff tiles
EPS = 1e-5
N_CORES = 8
VW = HS + 1            # 65: [v | ones] block per head
GB = 130               # a2a block rows: 128 attn feats + 2 denominators
SC = 0.125             # 1/sqrt(HS)


def build_program():
    nc = bacc.Bacc("TRN2", target_bir_lowering=False, debug=False,
                   enable_asserts=False, num_devices=N_CORES)

    d = {}
    d["xb"] = nc.dram_tensor("xb", (C, B * T), BF16, kind="ExternalInput").ap()
    d["xloc"] = nc.dram_tensor("xloc", (C, TL), F32, kind="ExternalInput").ap()
    d["wqkv"] = nc.dram_tensor("wqkv", (C, 384), BF16,
                               kind="ExternalInput").ap()
    d["wproj"] = nc.dram_tensor("wproj", (C, C), BF16,
                                kind="ExternalInput").ap()
    d["w1"] = nc.dram_tensor("w1", (C, FF), BF16, kind="ExternalInput").ap()
    d["w2"] = nc.dram_tensor("w2", (FF, C), BF16, kind="ExternalInput").ap()
    # bias/ln vectors arrive p-major [128, len/128] (host pre-transposes)
    for name, n in [("bproj", NCT), ("b1", NFT), ("b2", NCT), ("ln1g", NCT),
                    ("ln1b", NCT), ("ln2g", NCT), ("ln2b", NCT)]:
        d[name] = nc.dram_tensor(name, (128, n), F32,
                                 kind="ExternalInput").ap()
    d["maskpack"] = nc.dram_tensor("maskpack", (128, 4096), BF16,
                                   kind="ExternalInput").ap()
    d["ident"] = nc.dram_tensor("ident", (128, 128), BF16,
                                kind="ExternalInput").ap()
    d["selpack"] = nc.dram_tensor("selpack", (16, 8 * 128), BF16,
                                  kind="ExternalInput").ap()
    d["out"] = nc.dram_tensor("out", (C, TL), F32, kind="ExternalOutput").ap()

    with tile.TileContext(nc) as tc:
        _emit(tc, d)

    nc.compile()
    return nc


def _emit(tc, d):
    nc = tc.nc
    dmaq = [nc.sync, nc.scalar, nc.gpsimd]

    # ---------------- constants / small inputs ----------------
    const = tc.alloc_tile_pool(name="const", bufs=1)

    ident = const.tile([128, 128], BF16, tag="ident")
    nc.sync.dma_start(ident[:], d["ident"])

    # NOTE: the scalar queue carries NO DMAs until the FFN phases — a DMA
    # parked there (ring-credit waits) stalls every exp behind it.
    # Bias DMAs are emitted after the first x stripes (see below).
    bias_sb = {}
    for name in ("bproj", "b1", "b2", "ln1g", "ln1b", "ln2g", "ln2b"):
        n = d[name].shape[1]            # host passes p-major [128, n]
        t_ = const.tile([128, n], F32, tag=name, name=name)
        bias_sb[name] = t_
    bproj_sb, b1_sb, b2_sb = bias_sb["bproj"], bias_sb["b1"], bias_sb["b2"]
    ln1g_sb, ln1b_sb = bias_sb["ln1g"], bias_sb["ln1b"]
    ln2g_sb, ln2b_sb = bias_sb["ln2g"], bias_sb["ln2b"]

    ones_f32 = const.tile([128, 1], F32, tag="ones_f32")       # LN col-sum
    nc.gpsimd.memset(ones_f32[:], 1.0)
    eps_sb = const.tile([1, 1], F32, tag="eps")
    nc.gpsimd.memset(eps_sb[:], EPS)
    onesc = const.tile([1, 128], F32, tag="onesc")   # row-broadcast lhsT
    nc.gpsimd.memset(onesc[:], 1.0)
    maskpack = const.tile([128, 4096], BF16, tag="maskpack")
    selpack = const.tile([16, 8 * 128], BF16, tag="selpack")

    # ---------------- activation storage ----------------
    xloc_pool = tc.alloc_tile_pool(name="xloc_pool", bufs=1)
    xloc = [xloc_pool.tile([128, TL], F32, tag=f"xloc{i}", name=f"xloc{i}")
            for i in range(NCT)]
    wproj_pool = tc.alloc_tile_pool(name="wproj", bufs=1)
    wproj_sb = [wproj_pool.tile([128, C], BF16, tag=f"wp{i}", name=f"wp{i}")
                for i in range(NCT)]

    kqv_pool = tc.alloc_tile_pool(name="kqv_pool", bufs=1)
    q_sb = kqv_pool.tile([128, B * T], BF16, tag="q", name="q")
    k_sb = kqv_pool.tile([128, B * T], BF16, tag="k", name="k")
    v_sb = [kqv_pool.tile([128, 2 * VW], BF16, tag=f"v{j}", name=f"v{j}")
            for j in range(2 * (T // 128))]                    # 32 k-tiles
    wqkv_sb = [kqv_pool.tile([128, 384], BF16, tag=f"wqkv{i}",
                             name=f"wqkv{i}") for i in range(NCT)]
    vf_sb = kqv_pool.tile([128, T], BF16, tag="vf", name="vf")

    # right-side pools, bottom -> top (LIFO release order: top first)
    raw_pool = tc.alloc_tile_pool(name="raw_pool", bufs=1, side="right")
    attn_raw = [[raw_pool.tile([128, HT], BF16, tag=f"ar{b}_{s}",
                               name=f"ar{b}_{s}") for s in range(N_CORES)]
                for b in range(B)]
    den_raw = [raw_pool.tile([16, HT], BF16, tag=f"dr{b}", name=f"dr{b}")
               for b in range(B)]
    xbf1_pool = tc.alloc_tile_pool(name="xbf1_pool", bufs=1, side="right")
    xbf1 = [xbf1_pool.tile([128, T], BF16, tag=f"xb1_{i}", name=f"xb1_{i}")
            for i in range(NCT)]
    xbf0_pool = tc.alloc_tile_pool(name="xbf0_pool", bufs=1, side="right")
    xbf0 = [xbf0_pool.tile([128, T], BF16, tag=f"xb0_{i}", name=f"xb0_{i}")
            for i in range(NCT)]
    xbf = [xbf0, xbf1]

    # ---------------- input DMA, priority order ----------------
    # startup is HBM-bound AND the first exp waits on chunk-0's QKV: wqkv
    # then chunk-0/1 x stripes go out FIRST; everything else after.
    for i in range(NCT):
        [nc.sync, nc.gpsimd][i % 2].dma_start(
            wqkv_sb[i][:], d["wqkv"][i * 128:(i + 1) * 128, :])
    qi = 0

    def x_stripes(b, tcol):
        nonlocal qi
        for i in range(NCT):
            c0 = tcol * 1024
            [nc.sync, nc.gpsimd][qi % 2].dma_start(
                xbf[b][i][:, c0:c0 + 1024],
                d["xb"][i * 128:(i + 1) * 128,
                        b * T + c0:b * T + c0 + 1024])
            qi += 1

    x_stripes(0, 0)                             # chunks 0-1: unblock attn
    nc.sync.dma_start(maskpack[:], d["maskpack"])
    for k_, name in enumerate(bias_sb):
        [nc.gpsimd, nc.sync][k_ % 2].dma_start(bias_sb[name][:], d[name])
    x_stripes(0, 1)
    nc.sync.dma_start(selpack[:], d["selpack"])
    x_stripes(1, 0)
    x_stripes(1, 1)

    # a2a DRAM bounce buffers, no batch padding: group g on core p holds
    # p's 2 heads (+2 denom rows) for tokens [g*256,(g+1)*256) of batch b
    dram = tc.alloc_tile_pool(name="dram", bufs=1, space="DRAM")
    a2a_in = [dram.tile([N_CORES * GB, HT], BF16, tag=f"a2a_in{b}",
                        name=f"a2a_in{b}") for b in range(B)]
    a2a_out = [dram.tile([N_CORES * GB, HT], BF16, tag=f"a2a_out{b}",
                         name=f"a2a_out{b}") for b in range(B)]

    # the [v | ones] blocks' ones columns never change: fill them all now
    # (on vector — it is idle at startup; gpsimd is busy with x stripes)
    for j in range(2 * (T // 128)):
        vj = v_sb[j].rearrange("p (h w) -> p h w", w=VW)
        nc.vector.memset(vj[:, :, HS:VW], 1.0)

    # ---------------- PE clock warmup + keep-warm fillers ----------------
    # The HAM clock gate halves the PE clock after ~3.4us of idle and needs
    # ~3.4us of sustained activity to lift again. Dependency-free standalone
    # LDWEIGHTS on the identity (no PSUM, no consumers) emitted at known
    # bubble points keep the activity monitor busy.
    def filler(n):
        for _ in range(n):
            nc.tensor.ldweights(ident[:])

    with tc.tile_pool(name="warmps", bufs=1, space="PSUM") as wpool:
        wps = wpool.tile([128, 128], F32, tag="wps", name="wps")
        NWARM = 8
        for i in range(NWARM):
            nc.tensor.matmul(wps[:], ident[:], ident[:],
                             start=(i == 0), stop=(i == NWARM - 1))

    # ------------- P1+P2, chunk-interleaved, per batch + A2A -------------
    p2sb = tc.alloc_tile_pool(name="p2sb", bufs=1)
    warm = p2sb.tile([1, 1], F32, tag="warm", name="warm")
    nc.scalar.activation(warm[:], eps_sb[:], AF.Exp)

    p1ps = tc.alloc_tile_pool(name="p1ps", bufs=1, space="PSUM")    # 1 bank
    p1vps = tc.alloc_tile_pool(name="p1vps", bufs=1, space="PSUM")  # 1 bank
    p2ps = tc.alloc_tile_pool(name="p2ps", bufs=1, space="PSUM")    # 4 banks
    avps = tc.alloc_tile_pool(name="avps", bufs=1, space="PSUM")    # 2 banks

    def qkv_mms(b, tch):
        """Q/K/V matmuls for one 512-token chunk of batch b.

        Generator: yields after each matmul (True at part boundaries) so
        the caller can interleave these into attention's tensor idle
        slots. Transposes are NOT included — a PE transpose emitted while
        an attention accumulation group is open corrupts it.
        """
        for cols, dst, off in (
                (slice(0, 128), q_sb, b * T + tch * 512),
                (slice(128, 256), k_sb, b * T + tch * 512),
                (slice(256, 384), vf_sb, tch * 512)):
            ps = p1ps.tile([128, 512], F32, tag="p1", bufs=1, name="p1")
            for c in range(NCT):
                nc.tensor.matmul(ps[:], wqkv_sb[c][:, cols],
                                 xbf[b][c][:, tch * 512:(tch + 1) * 512],
                                 start=(c == 0), stop=(c == NCT - 1))
                if c < NCT - 1:
                    yield False
            nc.vector.tensor_copy(dst[:, off:off + 512], ps[:])
            yield True

    def v_transposes(b, tch):
        for kk in range(4):
            kt = 16 * b + 4 * tch + kk
            ps2 = p1vps.tile([128, 128], BF16, tag="p1v", bufs=1, name="p1v")
            nc.tensor.transpose(
                ps2[:], vf_sb[:, (4 * tch + kk) * 128:(4 * tch + kk + 1) * 128],
                ident[:])
            vj = v_sb[kt].rearrange("p (h w) -> p h w", w=VW)
            nc.vector.tensor_copy(
                vj[:, :, 0:HS], ps2[:].rearrange("p (h w) -> p h w", w=HS))

    # one global QKV stream: chunk (b,tch) = id 4b+tch; id 0 emitted
    # inline, ids 1..7 pumped into attention idle slots ACROSS batches
    # (batch 1's first chunks stream in during batch 0's last q-chunk).
    prog = [0]                                # highest fully-emitted id
    safe = [True]                             # gen at a part boundary?

    def qkv_all():
        for cid in range(1, 8):
            yield from qkv_mms(cid // 4, cid % 4)
            prog[0] = cid

    gen = qkv_all()

    def pump(n):
        for _ in range(n):
            r = next(gen, None)
            if r is None:
                prog[0] = 7
                safe[0] = True
                filler(1)
            else:
                safe[0] = r

    def attn_batch(b):
        """QKV + causal attention for the 2 local heads + A2A of batch b."""
        if b == 0:
            for _ in qkv_mms(0, 0):
                pass
        for j in range(4):                    # local q-chunks of 512
            while prog[0] < 4 * b + j or not safe[0]:
                pump(1)                       # chunk j emitted, group closed
            q0 = b * T + j * 512
            nkt = 4 * j + 4                   # causal k-tiles
            avs = [avps.tile([VW, 512], F32, tag=f"av{hh}", bufs=1,
                             name=f"av{hh}") for hh in range(2)]
            sc_t = [None] * nkt

            def emit_sc(kt):
                sc = p2ps.tile([128, 1024], F32, tag="sc", bufs=2,
                               name="sc")
                for hh, po in ((0, 0), (1, 64)):
                    nc.tensor.matmul(
                        sc[:, hh * 512:(hh + 1) * 512],
                        k_sb[po:po + HS,
                             b * T + kt * 128:b * T + (kt + 1) * 128],
                        q_sb[po:po + HS, q0:q0 + 512],
                        start=True, stop=True, tile_position=(po, 0))
                sc_t[kt] = sc

            emit_sc(0)
            # chunk j's v-transposes: after sc(kt0) so the first exp isn't
            # delayed, before av(kt0) so no accumulation group is open
            v_transposes(b, j)
            for kt in range(nkt):
                probs = p2sb.tile([128, 1024], BF16, tag="probs", bufs=6,
                                  name="probs")
                nc.scalar.activation(probs[:], sc_t[kt][:], AF.Exp, scale=SC)
                sc_t[kt] = None
                if kt + 1 < nkt:
                    emit_sc(kt + 1)
                if kt >= nkt - 4:             # diagonal k-tile: causal mask
                    i = kt - (nkt - 4)
                    nc.vector.tensor_mul(
                        probs[:], probs[:],
                        maskpack[:, i * 1024:(i + 1) * 1024])
                for hh in range(2):
                    nc.tensor.matmul(
                        avs[hh][:],
                        v_sb[b * 16 + kt][:, hh * VW:(hh + 1) * VW],
                        probs[:, hh * 512:(hh + 1) * 512],
                        start=(kt == 0), stop=(kt == nkt - 1))
                pump(2)
            # stage unnormalized attn + denominators into the A2A src.
            # bufs=4: every j gets its own slot, so this copy never waits
            # on staging DMAs that may be parked behind an in-flight A2A.
            for hh in range(2):
                sth = p2sb.tile([VW, 512], BF16, tag=f"st{hh}", bufs=4,
                                name=f"st{hh}")
                nc.vector.tensor_copy(sth[:], avs[hh][:])
                for s_ in range(2):
                    g = 2 * j + s_
                    csl = slice(s_ * HT, (s_ + 1) * HT)
                    nc.gpsimd.dma_start(
                        a2a_in[b][g * GB + hh * HS:g * GB + (hh + 1) * HS,
                                  :], sth[0:HS, csl])
                    nc.gpsimd.dma_start(
                        a2a_in[b][g * GB + 128 + hh:g * GB + 129 + hh, :],
                        sth[HS:VW, csl])
            pump(3)
        for _ in gen:                         # drain any leftover QKV
            pass
        nc.gpsimd.collective_compute(
            "AllToAll", ALU.bypass,
            replica_groups=[list(range(N_CORES))],
            ins=[a2a_in[b][:].opt()], outs=[a2a_out[b][:].opt()])

    attn_batch(0)
    xbf0_pool.release()
    # deferred loads the later phases need (HBM was saturated until here)
    for i in range(NCT):
        [nc.sync, nc.gpsimd][i % 2].dma_start(
            xloc[i][:], d["xloc"][i * 128:(i + 1) * 128, :])
    for i in range(NCT):
        [nc.sync, nc.gpsimd][i % 2].dma_start(
            wproj_sb[i][:], d["wproj"][i * 128:(i + 1) * 128, :])
    attn_batch(1)
    xbf1_pool.release()
    avps.release()
    p2ps.release()
    p1vps.release()
    p1ps.release()
    p2sb.release()
    kqv_pool.release()

    # ---------------- P3/P4 ----------------
    x2_pool = tc.alloc_tile_pool(name="x2_pool", bufs=1)
    x2b = [x2_pool.tile([128, TL], BF16, tag=f"x2b{i}", name=f"x2b{i}")
           for i in range(NCT)]
    h_pool = tc.alloc_tile_pool(name="h_pool", bufs=1)
    h_sb = [h_pool.tile([128, TL], BF16, tag=f"h{i}", name=f"h{i}")
            for i in range(NFT)]
    r2_pool = tc.alloc_tile_pool(name="r2_pool", bufs=1)
    resid2 = [r2_pool.tile([128, TL], F32, tag=f"r2_{i}", name=f"r2_{i}")
              for i in range(NCT)]

    # w1 resident (8MB) in the space freed by xbf; streamed in now
    w1res = tc.alloc_tile_pool(name="w1res", bufs=1, side="right")
    w1r = [w1res.tile([128, FF], BF16, tag=f"w1r{i}", name=f"w1r{i}")
           for i in range(NCT)]
    for qt in range(4):                       # quarter-major for FFN1a order
        for i in range(NCT):
            [nc.sync, nc.gpsimd][(qt * NCT + i) % 2].dma_start(
                w1r[i][:, qt * 1024:(qt + 1) * 1024],
                d["w1"][i * 128:(i + 1) * 128, qt * 1024:(qt + 1) * 1024])

    def p3_half(hf):
        """Normalize + project + residual + LN1 for one 256-token half."""
        cols = slice(hf * HT, (hf + 1) * HT)
        # gather this half's A2A result (the sync queue parks here on the
        # collective-done semaphore, so this is emitted as late as possible)
        for p in range(N_CORES):
            nc.sync.dma_start(attn_raw[hf][p][:],
                              a2a_out[hf][p * GB:p * GB + 128, :])
            nc.sync.dma_start(den_raw[hf][2 * p:2 * p + 2, :],
                              a2a_out[hf][p * GB + 128:(p + 1) * GB, :])
        filler(8)
        with tc.tile_pool(name=f"p3sb{hf}", bufs=1) as sb, \
             tc.tile_pool(name=f"p3ps{hf}", bufs=1, space="PSUM") as ps:
            rcpf = sb.tile([16, HT], F32, tag="rcpf", name="rcpf")
            nc.vector.reciprocal(rcpf[:], den_raw[hf][:])
            rcp16 = sb.tile([16, HT], BF16, tag="rcp16", name="rcp16")
            nc.vector.tensor_copy(rcp16[:], rcpf[:])
            attn_n = [sb.tile([128, HT], BF16, tag=f"an{s}", name=f"an{s}")
                      for s in range(N_CORES)]
            for s in range(N_CORES):
                bcp = ps.tile([128, HT], F32, tag="bc", bufs=2, name="bcp")
                nc.tensor.matmul(bcp[:], selpack[:, s * 128:(s + 1) * 128],
                                 rcp16[:], start=True, stop=True)
                nc.vector.tensor_mul(attn_n[s][:], attn_raw[hf][s][:],
                                     bcp[:])
                filler(1)

            resid1 = [sb.tile([128, HT], F32, tag=f"r1_{e}", name=f"r1_{e}")
                      for e in range(NCT)]
            mu_t = ps.tile([1, HT], F32, tag="mu", bufs=1, name="mu_t")
            sq_t = ps.tile([1, HT], F32, tag="sq", bufs=1, name="sq_t")
            mu_ps, sq_ps = mu_t[:], sq_t[:]
            for e in range(NCT):
                pr = ps.tile([128, HT], F32, tag="pr", bufs=2, name="pr")
                for s in range(NCT):
                    nc.tensor.matmul(
                        pr[:], wproj_sb[s][:, e * 128:(e + 1) * 128],
                        attn_n[s][:], start=(s == 0), stop=(s == NCT - 1))
                sa = sb.tile([128, HT], F32, tag="sa", bufs=2, name="sa")
                nc.vector.tensor_scalar_add(sa[:], pr[:], bproj_sb[:, e:e + 1])
                nc.gpsimd.tensor_add(resid1[e][:], sa[:], xloc[e][:, cols])
                # LN1 statistics, interleaved
                nc.tensor.matmul(mu_ps, ones_f32[:], resid1[e][:],
                                 start=(e == 0), stop=(e == NCT - 1))
                sqt = sb.tile([128, HT], F32, tag="sqt", bufs=2, name="sqt")
                eng = nc.vector if e % 2 else nc.gpsimd
                eng.tensor_mul(sqt[:], resid1[e][:], resid1[e][:])
                nc.tensor.matmul(sq_ps, ones_f32[:], sqt[:],
                                 start=(e == 0), stop=(e == NCT - 1))
                filler(2)
            # LN1 scalar chain on [1, 256]
            mu = sb.tile([1, HT], F32, tag="lnmu", name="lnmu")
            nc.scalar.activation(mu[:], mu_ps, AF.Identity, scale=1.0 / C)
            mu2 = sb.tile([1, HT], F32, tag="lnmu2", name="lnmu2")
            nc.scalar.square(mu2[:], mu[:])
            ms = sb.tile([1, HT], F32, tag="lnms", name="lnms")
            nc.scalar.activation(ms[:], sq_ps, AF.Identity, scale=1.0 / C)
            var = sb.tile([1, HT], F32, tag="lnvar", name="lnvar")
            nc.vector.tensor_sub(var[:], ms[:], mu2[:])
            sd = sb.tile([1, HT], F32, tag="lnsd", name="lnsd")
            nc.scalar.activation(sd[:], var[:], AF.Sqrt, bias=eps_sb[:])
            rstd = sb.tile([1, HT], F32, tag="lnrstd", name="lnrstd")
            nc.vector.reciprocal(rstd[:], sd[:])
            filler(16)
            mu_bcp = ps.tile([128, HT], F32, tag="bc", bufs=2, name="mubc")
            nc.tensor.matmul(mu_bcp[:], onesc[:], mu[:], start=True,
                             stop=True)
            rs_bcp = ps.tile([128, HT], F32, tag="bc", bufs=2, name="rsbc")
            nc.tensor.matmul(rs_bcp[:], onesc[:], rstd[:], start=True,
                             stop=True)
            for e in range(NCT):
                t1 = sb.tile([128, HT], F32, tag="t1", bufs=3, name="t1")
                nc.vector.tensor_sub(t1[:], resid1[e][:], mu_bcp[:])
                t2 = sb.tile([128, HT], F32, tag="t2", bufs=3, name="t2")
                nc.vector.tensor_mul(t2[:], t1[:], rs_bcp[:])
                # g*x + b on the (idle) scalar engine: per-partition scale
                nc.scalar.activation(x2b[e][:, cols], t2[:], AF.Identity,
                                     bias=ln1b_sb[:, e:e + 1],
                                     scale=ln1g_sb[:, e:e + 1])
                filler(3)

    def ffn1_half(hf):
        cols = slice(hf * HT, (hf + 1) * HT)
        with tc.tile_pool(name=f"f1ps{hf}", bufs=1, space="PSUM") as ps:
            for f in range(NFT):
                hp = ps.tile([128, HT], F32, tag="h1", bufs=4, name="h1")
                for c in range(NCT):
                    nc.tensor.matmul(hp[:], w1r[c][:, f * 128:(f + 1) * 128],
                                     x2b[c][:, cols],
                                     start=(c == 0), stop=(c == NCT - 1))
                nc.vector.tensor_scalar(h_sb[f][:, cols], hp[:],
                                        b1_sb[:, f:f + 1], 0.0,
                                        op0=ALU.add, op1=ALU.max)

    p3_half(0)
    ffn1_half(0)       # overlaps A2A#1
    p3_half(1)
    ffn1_half(1)
    w1res.release()

    # ---------------- FFN2 + LN2 ----------------
    p4w = tc.alloc_tile_pool(name="p4w_pool", bufs=1, side="right")
    statps = tc.alloc_tile_pool(name="statps", bufs=1, space="PSUM")
    mu2_ps = statps.tile([1, TL], F32, tag="mu2", name="mu2")
    sq2_ps = statps.tile([1, TL], F32, tag="sq2", name="sq2")
    with tc.tile_pool(name="p4sb", bufs=1) as sb4, \
         tc.tile_pool(name="p4ps", bufs=1, space="PSUM") as ps4:
        for ei in range(8):                    # eighths of FF
            w2e = [p4w.tile([128, C], BF16, tag=f"w2e{i}", bufs=2,
                            name=f"w2e{i}") for i in range(4)]
            for i in range(4):
                f = ei * 4 + i
                [nc.sync, nc.gpsimd][i % 2].dma_start(
                    w2e[i][:], d["w2"][f * 128:(f + 1) * 128, :])
            for e in range(NCT):
                ff = ps4.tile([128, TL], F32, tag="ff", bufs=3, name="ff")
                for i in range(4):
                    nc.tensor.matmul(ff[:], w2e[i][:, e * 128:(e + 1) * 128],
                                     h_sb[ei * 4 + i][:],
                                     start=(i == 0), stop=(i == 3))
                if ei == 0:
                    tmp = sb4.tile([128, TL], F32, tag="ft", bufs=3,
                                   name="ft")
                    nc.scalar.activation(tmp[:], ff[:], AF.Identity,
                                         bias=b2_sb[:, e:e + 1])
                    nc.vector.tensor_add(resid2[e][:], tmp[:], x2b[e][:])
                else:
                    nc.vector.tensor_add(resid2[e][:], resid2[e][:], ff[:])
                if ei == 7:
                    # LN2 statistics interleave with the last FFN2 pass
                    nc.tensor.matmul(mu2_ps[:], ones_f32[:], resid2[e][:],
                                     start=(e == 0), stop=(e == NCT - 1))
                    sq2t = sb4.tile([128, TL], F32, tag="sq2t", bufs=2,
                                    name="sq2t")
                    nc.scalar.square(sq2t[:], resid2[e][:])
                    nc.tensor.matmul(sq2_ps[:], ones_f32[:], sq2t[:],
                                     start=(e == 0), stop=(e == NCT - 1))
                    filler(4)

    # ---------------- LN2 + output ----------------
    with tc.tile_pool(name="p5sb", bufs=1) as sb5, \
         tc.tile_pool(name="p5ps", bufs=1, space="PSUM") as ps5:
        mu = sb5.tile([1, TL], F32, tag="lnmu", name="lnmu")
        nc.scalar.activation(mu[:], mu2_ps[:], AF.Identity, scale=1.0 / C)
        mu2 = sb5.tile([1, TL], F32, tag="lnmu2", name="lnmu2")
        nc.scalar.square(mu2[:], mu[:])
        ms = sb5.tile([1, TL], F32, tag="lnms", name="lnms")
        nc.scalar.activation(ms[:], sq2_ps[:], AF.Identity, scale=1.0 / C)
        var = sb5.tile([1, TL], F32, tag="lnvar", name="lnvar")
        nc.vector.tensor_sub(var[:], ms[:], mu2[:])
        sd = sb5.tile([1, TL], F32, tag="lnsd", name="lnsd")
        nc.scalar.activation(sd[:], var[:], AF.Sqrt, bias=eps_sb[:])
        rstd = sb5.tile([1, TL], F32, tag="lnrstd", name="lnrstd")
        nc.vector.reciprocal(rstd[:], sd[:])
        filler(24)
        mu_bcp = ps5.tile([128, TL], F32, tag="mubc", bufs=1, name="mubc")
        nc.tensor.matmul(mu_bcp[:], onesc[:], mu[:], start=True, stop=True)
        rs_bcp = ps5.tile([128, TL], F32, tag="rsbc", bufs=1, name="rsbc")
        nc.tensor.matmul(rs_bcp[:], onesc[:], rstd[:], start=True, stop=True)
        for e in range(NCT):
            t1 = sb5.tile([128, TL], F32, tag="t1", bufs=3, name="t1")
            nc.vector.tensor_sub(t1[:], resid2[e][:], mu_bcp[:])
            t2 = sb5.tile([128, TL], F32, tag="t2", bufs=3, name="t2")
            nc.vector.tensor_mul(t2[:], t1[:], rs_bcp[:])
            of = sb5.tile([128, TL], F32, tag="of", bufs=3, name="of")
            nc.scalar.activation(of[:], t2[:], AF.Identity,
                                 bias=ln2b_sb[:, e:e + 1],
                                 scale=ln2g_sb[:, e:e + 1])
            [nc.sync, nc.gpsimd][e % 2].dma_start(
                d["out"][e * 128:(e + 1) * 128, :], of[:])

    statps.release()
    r2_pool.release()
    h_pool.release()
    x2_pool.release()
    p4w.release()
    wproj_pool.release()
    raw_pool.release()
    dram.release()
    xloc_pool.release()
    const.release()


# ---------------- host side ----------------

def host_prepare(x, wq, wk, wv, wproj, bproj, ln1_g, ln1_b, w1, b1, w2, b2,
                 ln2_g, ln2_b):
    bf = ml_dtypes.bfloat16
    xT = np.concatenate([np.ascontiguousarray(x[b].T) for b in range(B)],
                        axis=1)                       # [C, B*T] fp32
    # causal masks for diagonal k-tiles, each duplicated for the 2 heads:
    # block i (cols [i*1024,(i+1)*1024)) = [m_i | m_i],
    # m_i[p, t] = 1 iff i*128 + p <= t
    p = np.arange(128)[:, None]
    t = np.arange(512)[None, :]
    mp = np.concatenate(
        [np.tile((128 * i + p <= t).astype(np.float32), (1, 2))
         for i in range(4)], axis=1)
    # selpack: sel_s[r, q] = 1 iff r == 2s + q//64 (head-denominator
    # broadcast: bcp[q, t] = rcp[2s + q//64, t])
    selpack = np.zeros((16, 8 * 128), np.float32)
    for s in range(8):
        for blk in range(2):
            selpack[2 * s + blk, s * 128 + blk * 64:s * 128 + (blk + 1) * 64] = 1
    def pmaj(v):     # [n*128] -> [128, n], row p holds v[p::128]... v[a*128+p]
        return np.ascontiguousarray(
            v.reshape(-1, 128).T).astype(np.float32)

    shared = {
        "xb": xT.astype(bf),
        "wproj": wproj.astype(bf),
        "w1": w1.astype(bf),
        "w2": w2.astype(bf),
        "bproj": pmaj(bproj),
        "b1": pmaj(b1),
        "b2": pmaj(b2),
        "ln1g": pmaj(ln1_g),
        "ln1b": pmaj(ln1_b),
        "ln2g": pmaj(ln2_g),
        "ln2b": pmaj(ln2_b),
        "maskpack": mp.astype(bf),
        "ident": np.eye(128).astype(bf),
        "selpack": selpack.astype(bf),
    }
    in_maps = []
    for core in range(N_CORES):
        h0 = 2 * core
        wqkv = np.concatenate(
            [wq[h0], wq[h0 + 1], wk[h0], wk[h0 + 1], wv[h0], wv[h0 + 1]],
            axis=1).astype(bf)                        # [C, 384]
        xloc = np.concatenate(
            [xT[:, b * T + core * HT: b * T + (core + 1) * HT]
             for b in range(B)], axis=1).astype(np.float32)   # [C, 512]
        in_maps.append({"wqkv": wqkv, "xloc": np.ascontiguousarray(xloc),
                        **shared})
    return in_maps


def host_finalize(results):
    out = np.empty((B, T, C), np.float32)
    for core in range(N_CORES):
        r = results[core]["out"]
        for b in range(B):
            out[b, core * HT:(core + 1) * HT, :] = \
                r[:, b * HT:(b + 1) * HT].T.astype(np.float32)
    return out


# ---------------- top-level entry ----------------
from concourse.bass_utils import run_bass_kernel_spmd as _run_spmd

_nc_cache = None


def _program():
    global _nc_cache
    if _nc_cache is None:
        _nc_cache = build_program()
    return _nc_cache


def run(inputs, trace=False):
    nc = _program()
    in_maps = host_prepare(**inputs)
    res = _run_spmd(nc, in_maps, core_ids=list(range(N_CORES)), trace=trace)
    return host_finalize(res.results), res


def kernel(**inputs):
    out, _ = run(inputs, trace=False)
    return out


# revision 81
# speedup vs baseline: 1.0933x; 1.0297x over previous
"""Self-contained Trainium2 Bass kernel for the dense transformer block.

Head-parallel attention + half-chunk token ownership:
 - Each of the 8 cores computes Q/K/V + causal attention for 2 of the 16
   heads over BOTH batch elements.
 - After batch b's attention, an 8-rank AllToAll redistributes the
   (unnormalized) attention outputs + softmax denominators so core c ends
   up owning tokens [c*256,(c+1)*256) of EVERY batch: 256 tokens from
   batch 0 (via A2A#0) and 256 from batch 1 (via A2A#1).
 - proj+LN1 run per 256-token half as soon as that half's A2A lands;
   FFN1 also runs per half (so the batch-0 half of the FFN overlaps
   A2A#1), while FFN2 + LN2 run once on the combined 512 columns.
Attention is exp/scalar-engine-bound (~1.5us per 128-k-tile ACTIVATE),
so everything else is arranged around keeping the scalar queue fed:
 - the scalar queue carries no DMAs before the FFN phases (a DMA parked
   there behind ring-credit waits stalls every exp);
 - all QKV matmuls stream through one global generator that is pumped,
   one matmul at a time, into attention's tensor idle slots across both
   batches (PE transposes stay at chunk boundaries: a transpose emitted
   while an accumulation group is open corrupts it);
 - A2A staging tiles get one slot per q-chunk so attention never waits
   on staging DMAs parked behind an in-flight collective, and the A2A
   gathers are emitted at P3 so they park only the idle sync queue;
 - standalone identity LDWEIGHTS at known bubble points keep the HAM
   clock gate from dropping the PE to half clock;
 - masks / identity / head-select matrices and p-major bias layouts are
   precomputed on host.
"""
import sys as _sys
if "/opt/trn_rl_repo" not in _sys.path:
    _sys.path.insert(0, "/opt/trn_rl_repo")

import numpy as np
import ml_dtypes

import concourse.bass as bass
import concourse.tile as tile
from concourse import bacc, mybir

F32 = mybir.dt.float32
BF16 = mybir.dt.bfloat16
AF = mybir.ActivationFunctionType
ALU = mybir.AluOpType

B, T, C, H, HS, FF = 2, 2048, 1024, 16, 64, 4096
TL = 512               # output columns per core (256 from each batch)
HT = 256               # tokens per core per batch
NCT = C // 128         # 8 feature tiles
NFT = FF // 128        # 32 ff tiles
EPS = 1e-5
N_CORES = 8
VW = HS + 1            # 65: [v | ones] block per head
GB = 130               # a2a block rows: 128 attn feats + 2 denominators
SC = 0.125             # 1/sqrt(HS)


def build_program():
    nc = bacc.Bacc("TRN2", target_bir_lowering=False, debug=False,
                   enable_asserts=False, num_devices=N_CORES)

    d = {}
    d["xb"] = nc.dram_tensor("xb", (C, B * T), BF16, kind="ExternalInput").ap()
    d["xloc"] = nc.dram_tensor("xloc", (C, TL), F32, kind="ExternalInput").ap()
    d["wqkv"] = nc.dram_tensor("wqkv", (C, 384), BF16,
                               kind="ExternalInput").ap()
    d["wproj"] = nc.dram_tensor("wproj", (C, C), BF16,
                                kind="ExternalInput").ap()
    d["w1"] = nc.dram_tensor("w1", (C, FF), BF16, kind="ExternalInput").ap()
    d["w2"] = nc.dram_tensor("w2", (FF, C), BF16, kind="ExternalInput").ap()
    # bias/ln vectors arrive p-major [128, len/128] (host pre-transposes)
    for name, n in [("bproj", NCT), ("b1", NFT), ("b2", NCT), ("ln1g", NCT),
                    ("ln1b", NCT), ("ln2g", NCT), ("ln2b", NCT)]:
        d[name] = nc.dram_tensor(name, (128, n), F32,
                                 kind="ExternalInput").ap()
    d["maskpack"] = nc.dram_tensor("maskpack", (128, 4096), BF16,
                                   kind="ExternalInput").ap()
    d["ident"] = nc.dram_tensor("ident", (128, 128), BF16,
                                kind="ExternalInput").ap()
    d["selpack"] = nc.dram_tensor("selpack", (16, 8 * 128), BF16,
                                  kind="ExternalInput").ap()
    d["out"] = nc.dram_tensor("out", (C, TL), F32, kind="ExternalOutput").ap()

    with tile.TileContext(nc) as tc:
        _emit(tc, d)

    nc.compile()
    return nc


def _emit(tc, d):
    nc = tc.nc
    dmaq = [nc.sync, nc.scalar, nc.gpsimd]

    # ---------------- constants / small inputs ----------------
    const = tc.alloc_tile_pool(name="const", bufs=1)

    ident = const.tile([128, 128], BF16, tag="ident")
    nc.sync.dma_start(ident[:], d["ident"])

    # NOTE: the scalar queue carries NO DMAs until the FFN phases — a DMA
    # parked there (ring-credit waits) stalls every exp behind it.
    # Bias DMAs are emitted after the first x stripes (see below).
    bias_sb = {}
    for name in ("bproj", "b1", "b2", "ln1g", "ln1b", "ln2g", "ln2b"):
        n = d[name].shape[1]            # host passes p-major [128, n]
        t_ = const.tile([128, n], F32, tag=name, name=name)
        bias_sb[name] = t_
    bproj_sb, b1_sb, b2_sb = bias_sb["bproj"], bias_sb["b1"], bias_sb["b2"]
    ln1g_sb, ln1b_sb = bias_sb["ln1g"], bias_sb["ln1b"]
    ln2g_sb, ln2b_sb = bias_sb["ln2g"], bias_sb["ln2b"]

    ones_f32 = const.tile([128, 1], F32, tag="ones_f32")       # LN col-sum
    nc.gpsimd.memset(ones_f32[:], 1.0)
    eps_sb = const.tile([1, 1], F32, tag="eps")
    nc.gpsimd.memset(eps_sb[:], EPS)
    onesc = const.tile([1, 128], F32, tag="onesc")   # row-broadcast lhsT
    nc.gpsimd.memset(onesc[:], 1.0)
    maskpack = const.tile([128, 4096], BF16, tag="maskpack")
    selpack = const.tile([16, 8 * 128], BF16, tag="selpack")

    # ---------------- activation storage ----------------
    xloc_pool = tc.alloc_tile_pool(name="xloc_pool", bufs=1)
    xloc = [xloc_pool.tile([128, TL], F32, tag=f"xloc{i}", name=f"xloc{i}")
            for i in range(NCT)]
    wproj_pool = tc.alloc_tile_pool(name="wproj", bufs=1)
    wproj_sb = [wproj_pool.tile([128, C], BF16, tag=f"wp{i}", name=f"wp{i}")
                for i in range(NCT)]

    kqv_pool = tc.alloc_tile_pool(name="kqv_pool", bufs=1)
    q_sb = kqv_pool.tile([128, B * T], BF16, tag="q", name="q")
    k_sb = kqv_pool.tile([128, B * T], BF16, tag="k", name="k")
    v_sb = [kqv_pool.tile([128, 2 * VW], BF16, tag=f"v{j}", name=f"v{j}")
            for j in range(2 * (T // 128))]                    # 32 k-tiles
    wqkv_sb = [kqv_pool.tile([128, 384], BF16, tag=f"wqkv{i}",
                             name=f"wqkv{i}") for i in range(NCT)]
    vf_sb = kqv_pool.tile([128, T], BF16, tag="vf", name="vf")

    # right-side pools, bottom -> top (LIFO release order: top first)
    raw_pool = tc.alloc_tile_pool(name="raw_pool", bufs=1, side="right")
    attn_raw = [[raw_pool.tile([128, HT], BF16, tag=f"ar{b}_{s}",
                               name=f"ar{b}_{s}") for s in range(N_CORES)]
                for b in range(B)]
    den_raw = [raw_pool.tile([16, HT], BF16, tag=f"dr{b}", name=f"dr{b}")
               for b in range(B)]
    xbf1_pool = tc.alloc_tile_pool(name="xbf1_pool", bufs=1, side="right")
    xbf1 = [xbf1_pool.tile([128, T], BF16, tag=f"xb1_{i}", name=f"xb1_{i}")
            for i in range(NCT)]
    xbf0_pool = tc.alloc_tile_pool(name="xbf0_pool", bufs=1, side="right")
    xbf0 = [xbf0_pool.tile([128, T], BF16, tag=f"xb0_{i}", name=f"xb0_{i}")
            for i in range(NCT)]
    xbf = [xbf0, xbf1]

    # ---------------- input DMA, priority order ----------------
    # startup is HBM-bound AND the first exp waits on chunk-0's QKV: wqkv
    # then chunk-0/1 x stripes go out FIRST; everything else after.
    for i in range(NCT):
        [nc.sync, nc.gpsimd][i % 2].dma_start(
            wqkv_sb[i][:], d["wqkv"][i * 128:(i + 1) * 128, :])
    qi = 0

    def x_stripes(b, tcol):
        nonlocal qi
        for i in range(NCT):
            c0 = tcol * 1024
            [nc.sync, nc.gpsimd][qi % 2].dma_start(
                xbf[b][i][:, c0:c0 + 1024],
                d["xb"][i * 128:(i + 1) * 128,
                        b * T + c0:b * T + c0 + 1024])
            qi += 1

    x_stripes(0, 0)                             # chunks 0-1: unblock attn
    nc.sync.dma_start(maskpack[:], d["maskpack"])
    for k_, name in enumerate(bias_sb):
        [nc.gpsimd, nc.sync][k_ % 2].dma_start(bias_sb[name][:], d[name])
    x_stripes(0, 1)
    nc.sync.dma_start(selpack[:], d["selpack"])
    x_stripes(1, 0)
    x_stripes(1, 1)

    # a2a DRAM bounce buffers, no batch padding: group g on core p holds
    # p's 2 heads (+2 denom rows) for tokens [g*256,(g+1)*256) of batch b
    dram = tc.alloc_tile_pool(name="dram", bufs=1, space="DRAM")
    a2a_in = [dram.tile([N_CORES * GB, HT], BF16, tag=f"a2a_in{b}",
                        name=f"a2a_in{b}") for b in range(B)]
    a2a_out = [dram.tile([N_CORES * GB, HT], BF16, tag=f"a2a_out{b}",
                         name=f"a2a_out{b}") for b in range(B)]

    # the [v | ones] blocks' ones columns never change: fill them all now
    # (on vector — it is idle at startup; gpsimd is busy with x stripes)
    for j in range(2 * (T // 128)):
        vj = v_sb[j].rearrange("p (h w) -> p h w", w=VW)
        nc.vector.memset(vj[:, :, HS:VW], 1.0)

    # ---------------- PE clock warmup + keep-warm fillers ----------------
    # The HAM clock gate halves the PE clock after ~3.4us of idle and needs
    # ~3.4us of sustained activity to lift again. Dependency-free standalone
    # LDWEIGHTS on the identity (no PSUM, no consumers) emitted at known
    # bubble points keep the activity monitor busy.
    def filler(n):
        for _ in range(n):
            nc.tensor.ldweights(ident[:])

    with tc.tile_pool(name="warmps", bufs=1, space="PSUM") as wpool:
        wps = wpool.tile([128, 128], F32, tag="wps", name="wps")
        NWARM = 8
        for i in range(NWARM):
            nc.tensor.matmul(wps[:], ident[:], ident[:],
                             start=(i == 0), stop=(i == NWARM - 1))

    # ------------- P1+P2, chunk-interleaved, per batch + A2A -------------
    p2sb = tc.alloc_tile_pool(name="p2sb", bufs=1)
    warm = p2sb.tile([1, 1], F32, tag="warm", name="warm")
    nc.scalar.activation(warm[:], eps_sb[:], AF.Exp)

    p1ps = tc.alloc_tile_pool(name="p1ps", bufs=1, space="PSUM")    # 1 bank
    p1vps = tc.alloc_tile_pool(name="p1vps", bufs=1, space="PSUM")  # 1 bank
    p2ps = tc.alloc_tile_pool(name="p2ps", bufs=1, space="PSUM")    # 4 banks
    avps = tc.alloc_tile_pool(name="avps", bufs=1, space="PSUM")    # 2 banks

    def qkv_mms(b, tch):
        """Q/K/V matmuls for one 512-token chunk of batch b.

        Generator: yields after each matmul (True at part boundaries) so
        the caller can interleave these into attention's tensor idle
        slots. Transposes are NOT included — a PE transpose emitted while
        an attention accumulation group is open corrupts it.
        """
        for cols, dst, off in (
                (slice(0, 128), q_sb, b * T + tch * 512),
                (slice(128, 256), k_sb, b * T + tch * 512),
                (slice(256, 384), vf_sb, tch * 512)):
            ps = p1ps.tile([128, 512], F32, tag="p1", bufs=1, name="p1")
            for c in range(NCT):
                nc.tensor.matmul(ps[:], wqkv_sb[c][:, cols],
                                 xbf[b][c][:, tch * 512:(tch + 1) * 512],
                                 start=(c == 0), stop=(c == NCT - 1))
                if c < NCT - 1:
                    yield False
            nc.vector.tensor_copy(dst[:, off:off + 512], ps[:])
            yield True

    def v_transposes(b, tch):
        for kk in range(4):
            kt = 16 * b + 4 * tch + kk
            ps2 = p1vps.tile([128, 128], BF16, tag="p1v", bufs=1, name="p1v")
            nc.tensor.transpose(
                ps2[:], vf_sb[:, (4 * tch + kk) * 128:(4 * tch + kk + 1) * 128],
                ident[:])
            vj = v_sb[kt].rearrange("p (h w) -> p h w", w=VW)
            nc.vector.tensor_copy(
                vj[:, :, 0:HS], ps2[:].rearrange("p (h w) -> p h w", w=HS))

    # one global QKV stream: chunk (b,tch) = id 4b+tch; id 0 emitted
    # inline, ids 1..7 pumped into attention idle slots ACROSS batches
    # (batch 1's first chunks stream in during batch 0's last q-chunk).
    prog = [0]                                # highest fully-emitted id
    safe = [True]                             # gen at a part boundary?

    def qkv_all():
        for cid in range(1, 8):
            yield from qkv_mms(cid // 4, cid % 4)
            prog[0] = cid

    gen = qkv_all()

    def pump(n):
        for _ in range(n):
            r = next(gen, None)
            if r is None:
                prog[0] = 7
                safe[0] = True
                filler(1)
            else:
                safe[0] = r

    def attn_batch(b):
        """QKV + causal attention for the 2 local heads + A2A of batch b."""
        if b == 0:
            for _ in qkv_mms(0, 0):
                pass
        for j in range(4):                    # local q-chunks of 512
            while prog[0] < 4 * b + j or not safe[0]:
                pump(1)                       # chunk j emitted, group closed
            q0 = b * T + j * 512
            nkt = 4 * j + 4                   # causal k-tiles
            avs = [avps.tile([VW, 512], F32, tag=f"av{hh}", bufs=1,
                             name=f"av{hh}") for hh in range(2)]
            sc_t = [None] * nkt

            def emit_sc(kt):
                sc = p2ps.tile([128, 1024], F32, tag="sc", bufs=2,
                               name="sc")
                for hh, po in ((0, 0), (1, 64)):
                    nc.tensor.matmul(
                        sc[:, hh * 512:(hh + 1) * 512],
                        k_sb[po:po + HS,
                             b * T + kt * 128:b * T + (kt + 1) * 128],
                        q_sb[po:po + HS, q0:q0 + 512],
                        start=True, stop=True, tile_position=(po, 0))
                sc_t[kt] = sc

            emit_sc(0)
            # chunk j's v-transposes: after sc(kt0) so the first exp isn't
            # delayed, before av(kt0) so no accumulation group is open
            v_transposes(b, j)
            for kt in range(nkt):
                probs = p2sb.tile([128, 1024], BF16, tag="probs", bufs=6,
                                  name="probs")
                nc.scalar.activation(probs[:], sc_t[kt][:], AF.Exp, scale=SC)
                sc_t[kt] = None
                if kt + 1 < nkt:
                    emit_sc(kt + 1)
                if kt >= nkt - 4:             # diagonal k-tile: causal mask
                    i = kt - (nkt - 4)
                    nc.vector.tensor_mul(
                        probs[:], probs[:],
                        maskpack[:, i * 1024:(i + 1) * 1024])
                for hh in range(2):
                    nc.tensor.matmul(
                        avs[hh][:],
                        v_sb[b * 16 + kt][:, hh * VW:(hh + 1) * VW],
                        probs[:, hh * 512:(hh + 1) * 512],
                        start=(kt == 0), stop=(kt == nkt - 1))
                pump(2)
            # stage unnormalized attn + denominators into the A2A src.
            # bufs=4: every j gets its own slot, so this copy never waits
            # on staging DMAs that may be parked behind an in-flight A2A.
            for hh in range(2):
                sth = p2sb.tile([VW, 512], BF16, tag=f"st{hh}", bufs=4,
                                name=f"st{hh}")
                nc.vector.tensor_copy(sth[:], avs[hh][:])
                for s_ in range(2):
                    g = 2 * j + s_
                    csl = slice(s_ * HT, (s_ + 1) * HT)
                    nc.gpsimd.dma_start(
                        a2a_in[b][g * GB + hh * HS:g * GB + (hh + 1) * HS,
                                  :], sth[0:HS, csl])
                    nc.gpsimd.dma_start(
                        a2a_in[b][g * GB + 128 + hh:g * GB + 129 + hh, :],
                        sth[HS:VW, csl])
            pump(3)
        for _ in gen:                         # drain any leftover QKV
            pass
        nc.gpsimd.collective_compute(
            "AllToAll", ALU.bypass,
            replica_groups=[list(range(N_CORES))],
            ins=[a2a_in[b][:].opt()], outs=[a2a_out[b][:].opt()])

    attn_batch(0)
    xbf0_pool.release()
    # deferred loads the later phases need (HBM was saturated until here)
    for i in range(NCT):
        [nc.sync, nc.gpsimd][i % 2].dma_start(
            xloc[i][:], d["xloc"][i * 128:(i + 1) * 128, :])
    for i in range(NCT):
        [nc.sync, nc.gpsimd][i % 2].dma_start(
            wproj_sb[i][:], d["wproj"][i * 128:(i + 1) * 128, :])
    attn_batch(1)
    xbf1_pool.release()
    avps.release()
    p2ps.release()
    p1vps.release()
    p1ps.release()
    p2sb.release()
    kqv_pool.release()

    # ---------------- P3/P4 ----------------
    x2_pool = tc.alloc_tile_pool(name="x2_pool", bufs=1)
    x2b = [x2_pool.tile([128, TL], BF16, tag=f"x2b{i}", name=f"x2b{i}")
           for i in range(NCT)]
    h_pool = tc.alloc_tile_pool(name="h_pool", bufs=1)
    h_sb = [h_pool.tile([128, TL], BF16, tag=f"h{i}", name=f"h{i}")
            for i in range(NFT)]
    r2_pool = tc.alloc_tile_pool(name="r2_pool", bufs=1)
    resid2 = [r2_pool.tile([128, TL], F32, tag=f"r2_{i}", name=f"r2_{i}")
              for i in range(NCT)]

    # w1 resident (8MB) in the space freed by xbf; streamed in now
    w1res = tc.alloc_tile_pool(name="w1res", bufs=1, side="right")
    w1r = [w1res.tile([128, FF], BF16, tag=f"w1r{i}", name=f"w1r{i}")
           for i in range(NCT)]
    for qt in range(4):                       # quarter-major for FFN1a order
        for i in range(NCT):
            [nc.sync, nc.gpsimd][(qt * NCT + i) % 2].dma_start(
                w1r[i][:, qt * 1024:(qt + 1) * 1024],
                d["w1"][i * 128:(i + 1) * 128, qt * 1024:(qt + 1) * 1024])

    def p3_half(hf):
        """Normalize + project + residual + LN1 for one 256-token half."""
        cols = slice(hf * HT, (hf + 1) * HT)
        # gather this half's A2A result (the sync queue parks here on the
        # collective-done semaphore, so this is emitted as late as possible)
        for p in range(N_CORES):
            nc.sync.dma_start(attn_raw[hf][p][:],
                              a2a_out[hf][p * GB:p * GB + 128, :])
            nc.sync.dma_start(den_raw[hf][2 * p:2 * p + 2, :],
                              a2a_out[hf][p * GB + 128:(p + 1) * GB, :])
        filler(8)
        with tc.tile_pool(name=f"p3sb{hf}", bufs=1) as sb, \
             tc.tile_pool(name=f"p3ps{hf}", bufs=1, space="PSUM") as ps:
            rcpf = sb.tile([16, HT], F32, tag="rcpf", name="rcpf")
            nc.vector.reciprocal(rcpf[:], den_raw[hf][:])
            rcp16 = sb.tile([16, HT], BF16, tag="rcp16", name="rcp16")
            nc.vector.tensor_copy(rcp16[:], rcpf[:])
            attn_n = [sb.tile([128, HT], BF16, tag=f"an{s}", name=f"an{s}")
                      for s in range(N_CORES)]
            for s in range(N_CORES):
                bcp = ps.tile([128, HT], F32, tag="bc", bufs=2, name="bcp")
                nc.tensor.matmul(bcp[:], selpack[:, s * 128:(s + 1) * 128],
                                 rcp16[:], start=True, stop=True)
                nc.vector.tensor_mul(attn_n[s][:], attn_raw[hf][s][:],
                                     bcp[:])
                filler(1)

            resid1 = [sb.tile([128, HT], F32, tag=f"r1_{e}", name=f"r1_{e}")
                      for e in range(NCT)]
            sqts = [sb.tile([128, HT], F32, tag=f"sqt{e}", name=f"sqt{e}")
                    for e in range(NCT)]
            mu_t = ps.tile([1, HT], F32, tag="mu", bufs=1, name="mu_t")
            sq_t = ps.tile([1, HT], F32, tag="sq", bufs=1, name="sq_t")
            mu_ps, sq_ps = mu_t[:], sq_t[:]
            for e in range(NCT):
                pr = ps.tile([128, HT], F32, tag="pr", bufs=2, name="pr")
                for s in range(NCT):
                    nc.tensor.matmul(
                        pr[:], wproj_sb[s][:, e * 128:(e + 1) * 128],
                        attn_n[s][:], start=(s == 0), stop=(s == NCT - 1))
                sa = sb.tile([128, HT], F32, tag="sa", bufs=2, name="sa")
                nc.vector.tensor_scalar_add(sa[:], pr[:], bproj_sb[:, e:e + 1])
                nc.gpsimd.tensor_add(resid1[e][:], sa[:], xloc[e][:, cols])
                eng = nc.vector if e % 2 else nc.gpsimd
                eng.tensor_mul(sqts[e][:], resid1[e][:], resid1[e][:])
                filler(2)
            # LN1 statistics in a second pass: an interleaved stats matmul
            # waiting on the sqt chain stalls the tensor FIFO mid-proj
            for e in range(NCT):
                nc.tensor.matmul(mu_ps, ones_f32[:], resid1[e][:],
                                 start=(e == 0), stop=(e == NCT - 1))
            for e in range(NCT):
                nc.tensor.matmul(sq_ps, ones_f32[:], sqts[e][:],
                                 start=(e == 0), stop=(e == NCT - 1))
            # LN1 scalar chain on [1, 256]
            mu = sb.tile([1, HT], F32, tag="lnmu", name="lnmu")
            nc.scalar.activation(mu[:], mu_ps, AF.Identity, scale=1.0 / C)
            mu2 = sb.tile([1, HT], F32, tag="lnmu2", name="lnmu2")
            nc.scalar.square(mu2[:], mu[:])
            ms = sb.tile([1, HT], F32, tag="lnms", name="lnms")
            nc.scalar.activation(ms[:], sq_ps, AF.Identity, scale=1.0 / C)
            var = sb.tile([1, HT], F32, tag="lnvar", name="lnvar")
            nc.vector.tensor_sub(var[:], ms[:], mu2[:])
            sd = sb.tile([1, HT], F32, tag="lnsd", name="lnsd")
            nc.scalar.activation(sd[:], var[:], AF.Sqrt, bias=eps_sb[:])
            rstd = sb.tile([1, HT], F32, tag="lnrstd", name="lnrstd")
            nc.vector.reciprocal(rstd[:], sd[:])
            filler(16)
            mu_bcp = ps.tile([128, HT], F32, tag="bc", bufs=2, name="mubc")
            nc.tensor.matmul(mu_bcp[:], onesc[:], mu[:], start=True,
                             stop=True)
            rs_bcp = ps.tile([128, HT], F32, tag="bc", bufs=2, name="rsbc")
            nc.tensor.matmul(rs_bcp[:], onesc[:], rstd[:], start=True,
                             stop=True)
            for e in range(NCT):
                t1 = sb.tile([128, HT], F32, tag="t1", bufs=3, name="t1")
                nc.vector.tensor_sub(t1[:], resid1[e][:], mu_bcp[:])
                t2 = sb.tile([128, HT], F32, tag="t2", bufs=3, name="t2")
                nc.vector.tensor_mul(t2[:], t1[:], rs_bcp[:])
                # g*x + b on the (idle) scalar engine: per-partition scale
                nc.scalar.activation(x2b[e][:, cols], t2[:], AF.Identity,
                                     bias=ln1b_sb[:, e:e + 1],
                                     scale=ln1g_sb[:, e:e + 1])
                filler(3)

    def ffn1_half(hf):
        cols = slice(hf * HT, (hf + 1) * HT)
        with tc.tile_pool(name=f"f1ps{hf}", bufs=1, space="PSUM") as ps:
            for f in range(NFT):
                hp = ps.tile([128, HT], F32, tag="h1", bufs=4, name="h1")
                for c in range(NCT):
                    nc.tensor.matmul(hp[:], w1r[c][:, f * 128:(f + 1) * 128],
                                     x2b[c][:, cols],
                                     start=(c == 0), stop=(c == NCT - 1))
                nc.vector.tensor_scalar(h_sb[f][:, cols], hp[:],
                                        b1_sb[:, f:f + 1], 0.0,
                                        op0=ALU.add, op1=ALU.max)

    p3_half(0)
    ffn1_half(0)       # overlaps A2A#1
    p3_half(1)
    ffn1_half(1)
    w1res.release()

    # ---------------- FFN2 + LN2 ----------------
    p4w = tc.alloc_tile_pool(name="p4w_pool", bufs=1, side="right")
    statps = tc.alloc_tile_pool(name="statps", bufs=1, space="PSUM")
    mu2_ps = statps.tile([1, TL], F32, tag="mu2", name="mu2")
    sq2_ps = statps.tile([1, TL], F32, tag="sq2", name="sq2")
    with tc.tile_pool(name="p4sb", bufs=1) as sb4, \
         tc.tile_pool(name="p4ps", bufs=1, space="PSUM") as ps4:
        for ei in range(8):                    # eighths of FF
            w2e = [p4w.tile([128, C], BF16, tag=f"w2e{i}", bufs=2,
                            name=f"w2e{i}") for i in range(4)]
            for i in range(4):
                f = ei * 4 + i
                [nc.sync, nc.gpsimd][i % 2].dma_start(
                    w2e[i][:], d["w2"][f * 128:(f + 1) * 128, :])
            for e in range(NCT):
                ff = ps4.tile([128, TL], F32, tag="ff", bufs=3, name="ff")
                for i in range(4):
                    nc.tensor.matmul(ff[:], w2e[i][:, e * 128:(e + 1) * 128],
                                     h_sb[ei * 4 + i][:],
                                     start=(i == 0), stop=(i == 3))
                if ei == 0:
                    tmp = sb4.tile([128, TL], F32, tag="ft", bufs=3,
                                   name="ft")
                    nc.scalar.activation(tmp[:], ff[:], AF.Identity,
                                         bias=b2_sb[:, e:e + 1])
                    nc.vector.tensor_add(resid2[e][:], tmp[:], x2b[e][:])
                else:
                    nc.vector.tensor_add(resid2[e][:], resid2[e][:], ff[:])
                if ei == 7:
                    # LN2 statistics interleave with the last FFN2 pass
                    nc.tensor.matmul(mu2_ps[:], ones_f32[:], resid2[e][:],
                                     start=(e == 0), stop=(e == NCT - 1))
                    sq2t = sb4.tile([128, TL], F32, tag="sq2t", bufs=2,
                                    name="sq2t")
                    nc.scalar.square(sq2t[:], resid2[e][:])
                    nc.tensor.matmul(sq2_ps[:], ones_f32[:], sq2t[:],
                                     start=(e == 0), stop=(e == NCT - 1))
                    filler(4)

    # ---------------- LN2 + output ----------------
    with tc.tile_pool(name="p5sb", bufs=1) as sb5, \
         tc.tile_pool(name="p5ps", bufs=1, space="PSUM") as ps5:
        mu = sb5.tile([1, TL], F32, tag="lnmu", name="lnmu")
        nc.scalar.activation(mu[:], mu2_ps[:], AF.Identity, scale=1.0 / C)
        mu2 = sb5.tile([1, TL], F32, tag="lnmu2", name="lnmu2")
        nc.scalar.square(mu2[:], mu[:])
        ms = sb5.tile([1, TL], F32, tag="lnms", name="lnms")
        nc.scalar.activation(ms[:], sq2_ps[:], AF.Identity, scale=1.0 / C)
        var = sb5.tile([1, TL], F32, tag="lnvar", name="lnvar")
        nc.vector.tensor_sub(var[:], ms[:], mu2[:])
        sd = sb5.tile([1, TL], F32, tag="lnsd", name="lnsd")
        nc.scalar.activation(sd[:], var[:], AF.Sqrt, bias=eps_sb[:])
        rstd = sb5.tile([1, TL], F32, tag="lnrstd", name="lnrstd")
        nc.vector.reciprocal(rstd[:], sd[:])
        filler(24)
        mu_bcp = ps5.tile([128, TL], F32, tag="mubc", bufs=1, name="mubc")
        nc.tensor.matmul(mu_bcp[:], onesc[:], mu[:], start=True, stop=True)
        rs_bcp = ps5.tile([128, TL], F32, tag="rsbc", bufs=1, name="rsbc")
        nc.tensor.matmul(rs_bcp[:], onesc[:], rstd[:], start=True, stop=True)
        # gpsimd can't read PSUM: stats copies so it can take two chains
        mu_bs = sb5.tile([128, TL], F32, tag="mubs", name="mubs")
        nc.vector.tensor_copy(mu_bs[:], mu_bcp[:])
        rs_bs = sb5.tile([128, TL], F32, tag="rsbs", name="rsbs")
        nc.vector.tensor_copy(rs_bs[:], rs_bcp[:])
        for e in range(NCT):
            eng = nc.gpsimd if e >= 6 else nc.vector
            t1 = sb5.tile([128, TL], F32, tag="t1", bufs=3, name="t1")
            eng.tensor_sub(t1[:], resid2[e][:], mu_bs[:])
            t2 = sb5.tile([128, TL], F32, tag="t2", bufs=3, name="t2")
            eng.tensor_mul(t2[:], t1[:], rs_bs[:])
            of = sb5.tile([128, TL], F32, tag="of", bufs=3, name="of")
            nc.scalar.activation(of[:], t2[:], AF.Identity,
                                 bias=ln2b_sb[:, e:e + 1],
                                 scale=ln2g_sb[:, e:e + 1])
            [nc.sync, nc.gpsimd][e % 2].dma_start(
                d["out"][e * 128:(e + 1) * 128, :], of[:])

    statps.release()
    r2_pool.release()
    h_pool.release()
    x2_pool.release()
    p4w.release()
    wproj_pool.release()
    raw_pool.release()
    dram.release()
    xloc_pool.release()
    const.release()


# ---------------- host side ----------------

def host_prepare(x, wq, wk, wv, wproj, bproj, ln1_g, ln1_b, w1, b1, w2, b2,
                 ln2_g, ln2_b):
    bf = ml_dtypes.bfloat16
    xT = np.concatenate([np.ascontiguousarray(x[b].T) for b in range(B)],
                        axis=1)                       # [C, B*T] fp32
    # causal masks for diagonal k-tiles, each duplicated for the 2 heads:
    # block i (cols [i*1024,(i+1)*1024)) = [m_i | m_i],
    # m_i[p, t] = 1 iff i*128 + p <= t
    p = np.arange(128)[:, None]
    t = np.arange(512)[None, :]
    mp = np.concatenate(
        [np.tile((128 * i + p <= t).astype(np.float32), (1, 2))
         for i in range(4)], axis=1)
    # selpack: sel_s[r, q] = 1 iff r == 2s + q//64 (head-denominator
    # broadcast: bcp[q, t] = rcp[2s + q//64, t])
    selpack = np.zeros((16, 8 * 128), np.float32)
    for s in range(8):
        for blk in range(2):
            selpack[2 * s + blk, s * 128 + blk * 64:s * 128 + (blk + 1) * 64] = 1
    def pmaj(v):     # [n*128] -> [128, n], row p holds v[p::128]... v[a*128+p]
        return np.ascontiguousarray(
            v.reshape(-1, 128).T).astype(np.float32)

    shared = {
        "xb": xT.astype(bf),
        "wproj": wproj.astype(bf),
        "w1": w1.astype(bf),
        "w2": w2.astype(bf),
        "bproj": pmaj(bproj),
        "b1": pmaj(b1),
        "b2": pmaj(b2),
        "ln1g": pmaj(ln1_g),
        "ln1b": pmaj(ln1_b),
        "ln2g": pmaj(ln2_g),
        "ln2b": pmaj(ln2_b),
        "maskpack": mp.astype(bf),
        "ident": np.eye(128).astype(bf),
        "selpack": selpack.astype(bf),
    }
    in_maps = []
    for core in range(N_CORES):
        h0 = 2 * core
        wqkv = np.concatenate(
            [wq[h0], wq[h0 + 1], wk[h0], wk[h0 + 1], wv[h0], wv[h0 + 1]],
            axis=1).astype(bf)                        # [C, 384]
        xloc = np.concatenate(
            [xT[:, b * T + core * HT: b * T + (core + 1) * HT]
             for b in range(B)], axis=1).astype(np.float32)   # [C, 512]
        in_maps.append({"wqkv": wqkv, "xloc": np.ascontiguousarray(xloc),
                        **shared})
    return in_maps


def host_finalize(results):
    out = np.empty((B, T, C), np.float32)
    for core in range(N_CORES):
        r = results[core]["out"]
        for b in range(B):
            out[b, core * HT:(core + 1) * HT, :] = \
                r[:, b * HT:(b + 1) * HT].T.astype(np.float32)
    return out


# ---------------- top-level entry ----------------
from concourse.bass_utils import run_bass_kernel_spmd as _run_spmd

_nc_cache = None


def _program():
    global _nc_cache
    if _nc_cache is None:
        _nc_cache = build_program()
    return _nc_cache


def run(inputs, trace=False):
    nc = _program()
    in_maps = host_prepare(**inputs)
    res = _run_spmd(nc, in_maps, core_ids=list(range(N_CORES)), trace=trace)
    return host_finalize(res.results), res


def kernel(**inputs):
    out, _ = run(inputs, trace=False)
    return out
